# revision 10
# baseline (speedup 1.0000x reference)
r"""Circulant layer kernel for Trainium2 (8 NeuronCores) — v4.

Math (same as v2/v3): reference computes mv1 + mv2 = 2 * circconv(d, b)
with d = des @ K, b = body @ K, realized by a real-input half-spectrum
DFT.  Cores 0..7 own freqs f = 64c..64c+63; the Nyquist f=512 rides
core 0's slot-0 imaginary column with the generalized 3-product (G3)
inverse folded into per-partition scales and index tables.

v4 on top of v3:
  * The trig constants (CC [1024,128], G3a/G3b [128,1024]) are GENERATED
    ON DEVICE instead of streamed from HBM (saves 768KB of the ~3.4MB
    per-core input stream, which runs at only ~300GB/s aggregate):
      gpsimd: iotas (j, s, jg) + int32 j*s + t2 broadcast add
      DVE:    (m+256)&1023 / (m+512)&1023 index ALU + int->f32 casts
              (gpsimd cannot do imm-scalar ALU or int->f32 converts)
      ACT:    Sin activations; sin(2pi m/1024) == Sin(pi - m*2pi/1024)
              keeps args in (-pi, pi] (the Sin table corrupts >~3pi/2)
      per-core specials (Nyquist rows/cols, +-4/N scales) ride a tiny
      aux DMA as per-partition scalars / override columns.
  * dbt (des^T|body^T) split across BOTH HWDGE queues between the kt
    k-halves; only the 10KB aux rides the slow SWDGE queue.
  * PSUM->SBUF copies all on DVE so ACT holds the Sin table (one swap
    back to the Copy table for the final cast, hidden in the stream).

One-sync-wait discipline (compute instrs encode exactly one wait):
DMA-landed operands are staged through a same-engine copy; the first
instruction of a cross-engine handoff carries the producer wait and
later ones are covered by escalating waits on the same semaphore.
"""

import numpy as np
import ml_dtypes

import concourse.bass as bass
import concourse.mybir as mybir
import concourse.tile as tile
from concourse.bass_utils import run_bass_kernel_spmd
from concourse.tile_rust import add_dep_helper

B = 128        # batch
D_IN = 1024    # input feature dim (contraction k)
N = 1024       # output feature dim (conv length j)
N_CORES = 8
FPC = 64       # complex frequency slots per core

F32 = mybir.dt.float32
I32 = mybir.dt.int32
BF16 = mybir.dt.bfloat16
SIN = mybir.ActivationFunctionType.Sin
ADD_ = mybir.AluOpType.add
AND_ = mybir.AluOpType.bitwise_and
MUL_ = mybir.AluOpType.mult
S2PI = float(-2.0 * np.pi / 1024.0)   # Sin(pi - m*2pi/1024) = sin(2pi m/1024)

# aux column map (i32 words; f32 values bitcast in cols 9..19)
AUX_T2 = 0      # [0:8)  (64*core*j) % 1024 per (p, chunk)
AUX_FG = 8      # f per partition for G3 rows
AUX_SA = 9      # G3a row scale (+-4/N, core-0 specials)
AUX_SB = 10     # G3b row scale (-4/N, core-0 zeros)
AUX_PI = 11     # pi (Sin bias)
AUX_IM0 = 12    # [12:20) cc im-slot-0 override values (f32)
AUX_W = 20

LAST_RESULT = None
_nc_cache = {}


def _build_nc():
    nc = bass.Bass(target_bir_lowering=True)

    # SP queue:  [id | kt h0 c0-3] [dbt c0-3] [kt h1 c0-3]
    # ACT queue: [kt h0 c4-7] [dbt c4-7] [kt h1 c4-7]
    # GP queue:  [aux (10KB)]
    sp1 = nc.declare_dram_parameter("sp1", [128, 64 + 1024], F32, False)
    spd = nc.declare_dram_parameter("spd", [128, 512], F32, False)
    sp2 = nc.declare_dram_parameter("sp2", [128, 1024], F32, False)
    ac1 = nc.declare_dram_parameter("ac1", [128, 1024], F32, False)
    acd = nc.declare_dram_parameter("acd", [128, 512], F32, False)
    ac2 = nc.declare_dram_parameter("ac2", [128, 1024], F32, False)
    aux = nc.declare_dram_parameter("aux", [128, AUX_W], I32, False)
    out = nc.declare_dram_parameter("out", [B, N // 2], F32, isOutput=True)

    with tile.TileContext(nc) as tc:
        with (
            tc.tile_pool(name="main", bufs=1) as pool,
            tc.tile_pool(name="psum", bufs=1, space="PSUM") as pp,
        ):
            sp1_sb = pool.tile([128, 64 + 1024], F32, tag="sp1", name="sp1")
            spd_sb = pool.tile([128, 512], F32, tag="spd", name="spd")
            sp2_sb = pool.tile([128, 1024], F32, tag="sp2", name="sp2")
            ac1_sb = pool.tile([128, 1024], F32, tag="ac1", name="ac1")
            acd_sb = pool.tile([128, 512], F32, tag="acd", name="acd")
            ac2_sb = pool.tile([128, 1024], F32, tag="ac2", name="ac2")
            aux_sb = pool.tile([128, AUX_W], I32, tag="aux", name="aux")

            in_dmas = []
            in_dmas.append(nc.sync.dma_start(sp1_sb[:], sp1[:, :]))
            in_dmas.append(nc.sync.dma_start(spd_sb[:], spd[:, :]))
            in_dmas.append(nc.sync.dma_start(sp2_sb[:], sp2[:, :]))
            in_dmas.append(nc.scalar.dma_start(ac1_sb[:], ac1[:, :]))
            in_dmas.append(nc.scalar.dma_start(acd_sb[:], acd[:, :]))
            in_dmas.append(nc.scalar.dma_start(ac2_sb[:], ac2[:, :]))
            in_dmas.append(nc.gpsimd.dma_start(aux_sb[:], aux[:, :]))

            # bf16 views
            id_v = sp1_sb.bitcast(BF16)[:, 0:128]
            ktv = {}
            for c in range(4):
                ktv[(c, 0)] = sp1_sb.bitcast(BF16)[:, 128 + c * 512:
                                                   128 + (c + 1) * 512]
                ktv[(c, 1)] = sp2_sb.bitcast(BF16)[:, c * 512:(c + 1) * 512]
                ktv[(4 + c, 0)] = ac1_sb.bitcast(BF16)[:, c * 512:(c + 1) * 512]
                ktv[(4 + c, 1)] = ac2_sb.bitcast(BF16)[:, c * 512:(c + 1) * 512]
            dbt_lo = spd_sb.bitcast(BF16).rearrange(
                "p (c w) -> p c w", c=4)          # [128, 4, 256] chunks 0-3
            dbt_hi = acd_sb.bitcast(BF16).rearrange(
                "p (c w) -> p c w", c=4)          # chunks 4-7
            aux_f = aux_sb.bitcast(F32)

            # ---- PSUM ----
            ps_kc0 = pp.tile([128, 512], F32, tag="pskc0", name="pskc0")
            ps_kc1 = pp.tile([128, 512], F32, tag="pskc1", name="pskc1")
            ps_db = pp.tile([128, 2 * B], F32, tag="psdb", name="psdb")
            trall = pp.tile([128, 4, 128], BF16, tag="trall", name="trall")
            trall2 = pp.tile([128, 4, 128], BF16, tag="trall2", name="trall2")
            ps_out_lo = pp.tile([128, 512], F32, tag="psoutl", name="psoutl")
            ps_out_hi = pp.tile([128, 512], F32, tag="psouth", name="psouth")

            # ================= constant generation =================
            # gpsimd: staging + iotas + cc m1 = j*s + t2
            t2c = pool.tile([128, 8], I32, tag="t2c", name="t2c")
            nc.gpsimd.tensor_copy(t2c[:], aux_sb[:, 0:8])     # waits aux DMA
            wz = pool.tile([128, 640], BF16, tag="wz", name="wz")
            memset_h = nc.gpsimd.memset(wz[:], 0.0)
            ji = pool.tile([128, 8, 64], I32, tag="ji", name="ji")
            nc.gpsimd.iota(ji[:], pattern=[[128, 8], [0, 64]],
                           base=0, channel_multiplier=1)
            si = pool.tile([128, 8, 64], I32, tag="si", name="si")
            nc.gpsimd.iota(si[:], pattern=[[0, 8], [1, 64]],
                           base=0, channel_multiplier=0)
            jg = pool.tile([128, 1024], I32, tag="jg", name="jg")
            nc.gpsimd.iota(jg[:], pattern=[[1, 1024]], base=0,
                           channel_multiplier=0)
            m0 = pool.tile([128, 8, 64], I32, tag="m0", name="m0")
            nc.gpsimd.tensor_tensor(m0[:], ji[:], si[:], op=MUL_)
            m1 = pool.tile([128, 8, 64], I32, tag="m1", name="m1")
            gp_last = nc.gpsimd.tensor_tensor(
                m1[:], m0[:], t2c[:].unsqueeze(2).broadcast_to([128, 8, 64]),
                op=ADD_)

            # DVE: staging, cc index ALU + casts first (unblocks ACT), then g3
            fgc = pool.tile([128, 1], I32, tag="fgc", name="fgc")
            nc.vector.tensor_copy(fgc[:], aux_sb[:, 8:9])     # waits aux DMA
            scl = pool.tile([128, 3], F32, tag="scl", name="scl")
            nc.vector.tensor_copy(scl[:], aux_f[:, 9:12])     # sa, sb, pi
            im0 = pool.tile([128, 8], BF16, tag="im0", name="im0")
            nc.vector.tensor_copy(im0[:], aux_f[:, 12:20])
            mca = pool.tile([128, 8, 64], I32, tag="mca", name="mca")
            nc.vector.tensor_scalar(mca[:], m1[:], 256, None, op0=ADD_)
            nc.vector.tensor_scalar(mca[:], mca[:], 1023, None, op0=AND_)
            msa = pool.tile([128, 8, 64], I32, tag="msa", name="msa")
            nc.vector.tensor_scalar(msa[:], m1[:], 512, None, op0=ADD_)
            nc.vector.tensor_scalar(msa[:], msa[:], 1023, None, op0=AND_)
            mcf = pool.tile([128, 8, 64], F32, tag="mcf", name="mcf")
            nc.vector.tensor_copy(mcf[:], mca[:])
            msf = pool.tile([128, 8, 64], F32, tag="msf", name="msf")
            nc.vector.tensor_copy(msf[:], msa[:])
            # g3 indices (jg staged onto DVE so m2 needs one self-wait)
            jgd = pool.tile([128, 1024], I32, tag="jgd", name="jgd")
            nc.vector.tensor_copy(jgd[:], jg[:])
            m2 = pool.tile([128, 1024], I32, tag="m2", name="m2")
            nc.vector.tensor_tensor(m2[:], jgd[:],
                                    fgc[:].broadcast_to([128, 1024]), op=MUL_)
            ga_i = pool.tile([128, 1024], I32, tag="gai", name="gai")
            nc.vector.tensor_scalar(ga_i[:], m2[:], 256, None, op0=ADD_)
            nc.vector.tensor_scalar(ga_i[:], ga_i[:], 1023, None, op0=AND_)
            gs_i = pool.tile([128, 1024], I32, tag="gsi", name="gsi")
            nc.vector.tensor_scalar(gs_i[:], m2[:], 512, None, op0=ADD_)
            nc.vector.tensor_scalar(gs_i[:], gs_i[:], 1023, None, op0=AND_)
            gaf = pool.tile([128, 1024], F32, tag="gaf", name="gaf")
            nc.vector.tensor_copy(gaf[:], ga_i[:])
            gsf = pool.tile([128, 1024], F32, tag="gsf", name="gsf")
            nc.vector.tensor_copy(gsf[:], gs_i[:])

            # ACT: four Sins (single Sin-table residency)
            cc_raw = pool.tile([128, 8, 128], BF16, tag="ccraw", name="ccraw")
            nc.scalar.activation(cc_raw[:, :, 0:64], mcf[:], SIN,
                                 bias=scl[:, 2:3], scale=S2PI)
            nc.scalar.activation(cc_raw[:, :, 64:128], msf[:], SIN,
                                 bias=scl[:, 2:3], scale=S2PI)
            g3a_raw = pool.tile([128, 1024], BF16, tag="g3ar", name="g3ar")
            nc.scalar.activation(g3a_raw[:], gaf[:], SIN,
                                 bias=scl[:, 2:3], scale=S2PI)
            g3b_raw = pool.tile([128, 1024], BF16, tag="g3br", name="g3br")
            act_g_last = nc.scalar.activation(g3b_raw[:], gsf[:], SIN,
                                              bias=scl[:, 2:3], scale=S2PI)

            # DVE: finalize cc (single producer for the PE wait) + g3 scales
            cc_t = pool.tile([128, 8, 128], BF16, tag="cct", name="cct")
            nc.vector.tensor_copy(
                cc_t[:].rearrange("p c s -> p (c s)"),
                cc_raw[:].rearrange("p c s -> p (c s)"))
            nc.vector.tensor_copy(cc_t[:, :, 64:65],
                                  im0[:].unsqueeze(2))
            g3a_v = pool.tile([128, 1024], BF16, tag="g3a", name="g3a")
            nc.vector.tensor_scalar(g3a_v[:], g3a_raw[:], scl[:, 0:1], None,
                                    op0=MUL_)
            g3b_v = pool.tile([128, 1024], BF16, tag="g3b", name="g3b")
            nc.vector.tensor_scalar(g3b_v[:], g3b_raw[:], scl[:, 1:2], None,
                                    op0=MUL_)

            # ================= main pipeline =================
            # PE warmup junk into ps_out (S4 start=True overwrites)
            for w in range(4):
                nc.tensor.matmul(ps_out_lo[:], wz[:, :128], wz[:, 128:640],
                                 start=True, stop=True)

            # S1 h0
            h_order = [0, 1, 4, 5, 2, 3, 6, 7]
            for i, c in enumerate(h_order):
                nc.tensor.matmul(ps_kc0[:], cc_t[:, c, :], ktv[(c, 0)],
                                 start=(i == 0), stop=(i == 7))
            # T1 h0 (kcT copy on DVE, transposes on PE, kc copy on DVE)
            kcT0 = pool.tile([128, 512], BF16, tag="kcT0", name="kcT0")
            nc.vector.tensor_copy(kcT0[:], ps_kc0[:])
            for c in range(4):
                nc.tensor.transpose(trall[:, c, :],
                                    kcT0[:, c * 128:(c + 1) * 128], id_v)
            kc_lo = pool.tile([128, 4, 128], BF16, tag="kclo", name="kclo")
            nc.vector.tensor_copy(
                kc_lo[:].rearrange("p c s -> p (c s)"),
                trall[:].rearrange("p c s -> p (c s)"))

            # S1 h1 first pair + S2 partial 0 + S1 h1 rest
            for i, c in enumerate(h_order[:2]):
                nc.tensor.matmul(ps_kc1[:], cc_t[:, c, :], ktv[(c, 1)],
                                 start=(i == 0), stop=False)
            for c in range(4):
                nc.tensor.matmul(ps_db[:], kc_lo[:, c, :], dbt_lo[:, c, :],
                                 start=(c == 0), stop=False)
            for i, c in enumerate(h_order[2:]):
                nc.tensor.matmul(ps_kc1[:], cc_t[:, c, :], ktv[(c, 1)],
                                 start=False, stop=(i == 5))

            # T1 h1
            kcT1 = pool.tile([128, 512], BF16, tag="kcT1", name="kcT1")
            nc.vector.tensor_copy(kcT1[:], ps_kc1[:])
            for c in range(4):
                nc.tensor.transpose(trall2[:, c, :],
                                    kcT1[:, c * 128:(c + 1) * 128], id_v)
            kc_hi = pool.tile([128, 4, 128], BF16, tag="kchi", name="kchi")
            nc.vector.tensor_copy(
                kc_hi[:].rearrange("p c s -> p (c s)"),
                trall2[:].rearrange("p c s -> p (c s)"))

            # S2 partial 1
            for c in range(4):
                nc.tensor.matmul(ps_db[:], kc_hi[:, c, :], dbt_hi[:, c, :],
                                 start=False, stop=(c == 3))

            # PW in [s, b] layout
            db_sb = pool.tile([128, 2 * B], BF16, tag="db", name="db")
            nc.vector.tensor_copy(db_sb[:], ps_db[:])
            dbsw = pool.tile([128, B], BF16, tag="dbsw", name="dbsw")
            nc.vector.tensor_copy(dbsw[0:64, :], db_sb[64:128, B:2 * B])
            nc.vector.tensor_copy(dbsw[64:128, :], db_sb[0:64, B:2 * B])
            ptA = pool.tile([128, B], BF16, tag="ptA", name="ptA")
            ptC2 = pool.tile([128, B], BF16, tag="ptC2", name="ptC2")
            nc.vector.tensor_mul(ptA[:], db_sb[:, 0:B], db_sb[:, B:2 * B])
            nc.vector.tensor_mul(ptC2[:], db_sb[:, 0:B], dbsw[:])

            # S4 per bank + cast + store
            out_lo = pool.tile([128, 512], BF16, tag="outlo", name="outlo")
            out_hi = pool.tile([128, 512], BF16, tag="outhi", name="outhi")
            stores = []
            nc.tensor.matmul(ps_out_lo[:], ptA[:], g3a_v[:, 0:512],
                             start=True, stop=False)
            nc.tensor.matmul(ps_out_lo[:], ptC2[:], g3b_v[:, 0:512],
                             start=False, stop=True)
            cp_lo = nc.scalar.copy(out_lo[:], ps_out_lo[:])
            stores.append(nc.sync.dma_start(out[:, :256],
                                            out_lo.bitcast(F32)[:, :]))
            nc.tensor.matmul(ps_out_hi[:], ptA[:], g3a_v[:, 512:1024],
                             start=True, stop=False)
            last_mm = nc.tensor.matmul(ps_out_hi[:], ptC2[:],
                                       g3b_v[:, 512:1024],
                                       start=False, stop=True)
            cp_hi = nc.vector.tensor_copy(out_hi[:], ps_out_hi[:])
            stores.append(nc.scalar.dma_start(out[:, 256:],
                                              out_hi.bitcast(F32)[:, :]))

            # tail: absorb every outstanding tick into SP's clock
            prev = None
            for dep in [*in_dmas, memset_h, gp_last, act_g_last, *stores,
                        last_mm, cp_lo, cp_hi]:
                dr = nc.sync.drain(fusable=False)
                add_dep_helper(dr.ins, dep.ins, sync=True,
                               reason="tail: absorb tick into SP clock")
                if prev is not None:
                    add_dep_helper(dr.ins, prev.ins, sync=False,
                                   reason="tail: keep drain chain ordered")
                prev = dr

    return nc


def _bf16_pack(a):
    """float32 (P, W) -> bf16 packed two-per-word as float32 (P, W//2)."""
    bf = np.ascontiguousarray(np.asarray(a, np.float32).astype(ml_dtypes.bfloat16))
    return bf.view(np.uint8).reshape(bf.shape[0], -1).view(np.float32)


def _partition_pack(a):
    """(n*128, W) -> (128, n*W): row p = concat of chunk rows p."""
    r, w = a.shape
    n = r // 128
    return np.ascontiguousarray(
        a.reshape(n, 128, w).transpose(1, 0, 2).reshape(128, n * w))


def _aux_for_core(core):
    p = np.arange(128)
    c = np.arange(8)
    j = c[None, :] * 128 + p[:, None]                 # [128, 8]
    auxm = np.zeros((128, AUX_W), np.int32)
    auxm[:, 0:8] = (64 * core * j) % 1024
    fg = 64 * core + (p % 64)
    sa = np.where(p < 64, 4.0 / N, -4.0 / N).astype(np.float32)
    sb = np.full(128, -4.0 / N, np.float32)
    # cc im slot-0 override: normal value is -sin(2pi j (64 core)/1024);
    # core 0 carries the Nyquist cos(pi j) = (-1)^j column instead.
    if core == 0:
        im0 = np.cos(np.pi * j).astype(np.float32)
        fg = fg.copy()
        fg[64] = 512                 # Bm row 0 -> (2/N) cos(pi j)
        sa = sa.copy()
        sa[0] = 2.0 / N              # A row 0 -> 2/N constant (cos(0)=1)
        sa[64] = 2.0 / N             # Bm row 0 sign+scale
        sb = sb.copy()
        sb[0] = 0.0                  # C rows 0 and dup are zero
        sb[64] = 0.0
    else:
        im0 = -np.sin(2.0 * np.pi * j * (64 * core) / N).astype(np.float32)
    fv = np.zeros((128, AUX_W), np.float32)
    fv[:, AUX_SA] = sa
    fv[:, AUX_SB] = sb
    fv[:, AUX_PI] = np.pi
    fv[:, AUX_IM0:AUX_IM0 + 8] = im0
    auxm[:, AUX_FG] = fg
    outm = auxm.copy()
    outm[:, AUX_SA:] = fv[:, AUX_SA:].view(np.int32)
    return np.ascontiguousarray(outm)


def kernel(des, body, kernel):
    global LAST_RESULT
    K = np.asarray(kernel, dtype=np.float32)
    des = np.asarray(des, dtype=np.float32)
    body = np.asarray(body, dtype=np.float32)

    ktb = K.T.astype(ml_dtypes.bfloat16)                # (1024 j, 1024 k)
    def ktpk(c, h):
        blk = np.ascontiguousarray(
            ktb[c * 128:(c + 1) * 128, h * 512:(h + 1) * 512], np.float32)
        return _bf16_pack(blk)                          # (128, 256) words

    id_pk = _bf16_pack(np.eye(128, dtype=np.float32))   # (128, 64) words
    dbt_np = np.concatenate([des.T, body.T], axis=1)    # (1024, 256)
    dbt_pk = _partition_pack(_bf16_pack(dbt_np))        # (128, 1024) words

    sp1v = np.ascontiguousarray(np.concatenate(
        [id_pk] + [ktpk(c, 0) for c in range(4)], axis=1))
    spdv = np.ascontiguousarray(dbt_pk[:, 0:512])
    sp2v = np.ascontiguousarray(np.concatenate(
        [ktpk(c, 1) for c in range(4)], axis=1))
    ac1v = np.ascontiguousarray(np.concatenate(
        [ktpk(4 + c, 0) for c in range(4)], axis=1))
    acdv = np.ascontiguousarray(dbt_pk[:, 512:1024])
    ac2v = np.ascontiguousarray(np.concatenate(
        [ktpk(4 + c, 1) for c in range(4)], axis=1))

    in_maps = []
    for core in range(N_CORES):
        in_maps.append({
            "sp1": sp1v, "spd": spdv, "sp2": sp2v,
            "ac1": ac1v, "acd": acdv, "ac2": ac2v,
            "aux": _aux_for_core(core),
        })

    if "nc" not in _nc_cache:
        _nc_cache["nc"] = _build_nc()
    nc = _nc_cache["nc"]

    res = run_bass_kernel_spmd(nc, in_maps, list(range(N_CORES)))
    LAST_RESULT = res
    out = np.zeros((B, N), dtype=np.float32)
    for r in res.results:
        w = np.ascontiguousarray(np.asarray(r["out"], np.float32))
        bf = w.view(np.uint8).reshape(B, -1).view(ml_dtypes.bfloat16)
        out += bf.astype(np.float32)
    return out


# revision 11
# speedup vs baseline: 1.0079x; 1.0079x over previous
r"""Circulant layer kernel for Trainium2 (8 NeuronCores) — v4.

Math (same as v2/v3): reference computes mv1 + mv2 = 2 * circconv(d, b)
with d = des @ K, b = body @ K, realized by a real-input half-spectrum
DFT.  Cores 0..7 own freqs f = 64c..64c+63; the Nyquist f=512 rides
core 0's slot-0 imaginary column with the generalized 3-product (G3)
inverse folded into per-partition scales and index tables.

v4 on top of v3:
  * The trig constants (CC [1024,128], G3a/G3b [128,1024]) are GENERATED
    ON DEVICE instead of streamed from HBM (saves 768KB of the ~3.4MB
    per-core input stream, which runs at only ~300GB/s aggregate):
      gpsimd: iotas (j, s, jg) + int32 j*s + t2 broadcast add
      DVE:    (m+256)&1023 / (m+512)&1023 index ALU + int->f32 casts
              (gpsimd cannot do imm-scalar ALU or int->f32 converts)
      ACT:    Sin activations; sin(2pi m/1024) == Sin(pi - m*2pi/1024)
              keeps args in (-pi, pi] (the Sin table corrupts >~3pi/2)
      per-core specials (Nyquist rows/cols, +-4/N scales) ride a tiny
      aux DMA as per-partition scalars / override columns.
  * dbt (des^T|body^T) split across BOTH HWDGE queues between the kt
    k-halves; only the 10KB aux rides the slow SWDGE queue.
  * PSUM->SBUF copies all on DVE so ACT holds the Sin table (one swap
    back to the Copy table for the final cast, hidden in the stream).

One-sync-wait discipline (compute instrs encode exactly one wait):
DMA-landed operands are staged through a same-engine copy; the first
instruction of a cross-engine handoff carries the producer wait and
later ones are covered by escalating waits on the same semaphore.
"""

import numpy as np
import ml_dtypes

import concourse.bass as bass
import concourse.mybir as mybir
import concourse.tile as tile
from concourse.bass_utils import run_bass_kernel_spmd
from concourse.tile_rust import add_dep_helper

B = 128        # batch
D_IN = 1024    # input feature dim (contraction k)
N = 1024       # output feature dim (conv length j)
N_CORES = 8
FPC = 64       # complex frequency slots per core

F32 = mybir.dt.float32
I32 = mybir.dt.int32
BF16 = mybir.dt.bfloat16
SIN = mybir.ActivationFunctionType.Sin
ADD_ = mybir.AluOpType.add
AND_ = mybir.AluOpType.bitwise_and
MUL_ = mybir.AluOpType.mult
S2PI = float(-2.0 * np.pi / 1024.0)   # Sin(pi - m*2pi/1024) = sin(2pi m/1024)

# aux column map (i32 words; f32 values bitcast in cols 9..19)
AUX_T2 = 0      # [0:8)  (64*core*j) % 1024 per (p, chunk)
AUX_FG = 8      # f per partition for G3 rows
AUX_SA = 9      # G3a row scale (+-4/N, core-0 specials)
AUX_SB = 10     # G3b row scale (-4/N, core-0 zeros)
AUX_PI = 11     # pi (Sin bias)
AUX_IM0 = 12    # [12:20) cc im-slot-0 override values (f32)
AUX_W = 20

LAST_RESULT = None
_nc_cache = {}


def _build_nc():
    nc = bass.Bass(target_bir_lowering=True)

    # SP queue:  [id | kt h0 c0-3] [dbt c0-3] [kt h1 c0-3]
    # ACT queue: [kt h0 c4-7] [dbt c4-7] [kt h1 c4-7]
    # GP queue:  [aux (10KB)]
    sp1 = nc.declare_dram_parameter("sp1", [128, 64 + 1024], F32, False)
    spd = nc.declare_dram_parameter("spd", [128, 512], F32, False)
    sp2 = nc.declare_dram_parameter("sp2", [128, 1024], F32, False)
    ac1 = nc.declare_dram_parameter("ac1", [128, 1024], F32, False)
    acd = nc.declare_dram_parameter("acd", [128, 512], F32, False)
    ac2 = nc.declare_dram_parameter("ac2", [128, 1024], F32, False)
    aux = nc.declare_dram_parameter("aux", [128, AUX_W], I32, False)
    out = nc.declare_dram_parameter("out", [B, N // 2], F32, isOutput=True)

    with tile.TileContext(nc) as tc:
        with (
            tc.tile_pool(name="main", bufs=1) as pool,
            tc.tile_pool(name="psum", bufs=1, space="PSUM") as pp,
        ):
            sp1_sb = pool.tile([128, 64 + 1024], F32, tag="sp1", name="sp1")
            spd_sb = pool.tile([128, 512], F32, tag="spd", name="spd")
            sp2_sb = pool.tile([128, 1024], F32, tag="sp2", name="sp2")
            ac1_sb = pool.tile([128, 1024], F32, tag="ac1", name="ac1")
            acd_sb = pool.tile([128, 512], F32, tag="acd", name="acd")
            ac2_sb = pool.tile([128, 1024], F32, tag="ac2", name="ac2")
            aux_sb = pool.tile([128, AUX_W], I32, tag="aux", name="aux")

            in_dmas = []
            in_dmas.append(nc.sync.dma_start(sp1_sb[:], sp1[:, :]))
            in_dmas.append(nc.sync.dma_start(spd_sb[:], spd[:, :]))
            in_dmas.append(nc.sync.dma_start(sp2_sb[:], sp2[:, :]))
            in_dmas.append(nc.scalar.dma_start(ac1_sb[:], ac1[:, :]))
            in_dmas.append(nc.scalar.dma_start(acd_sb[:], acd[:, :]))
            in_dmas.append(nc.scalar.dma_start(ac2_sb[:], ac2[:, :]))
            in_dmas.append(nc.gpsimd.dma_start(aux_sb[:], aux[:, :]))

            # bf16 views
            id_v = sp1_sb.bitcast(BF16)[:, 0:128]
            ktv = {}
            for c in range(4):
                ktv[(c, 0)] = sp1_sb.bitcast(BF16)[:, 128 + c * 512:
                                                   128 + (c + 1) * 512]
                ktv[(c, 1)] = sp2_sb.bitcast(BF16)[:, c * 512:(c + 1) * 512]
                ktv[(4 + c, 0)] = ac1_sb.bitcast(BF16)[:, c * 512:(c + 1) * 512]
                ktv[(4 + c, 1)] = ac2_sb.bitcast(BF16)[:, c * 512:(c + 1) * 512]
            dbt_lo = spd_sb.bitcast(BF16).rearrange(
                "p (c w) -> p c w", c=4)          # [128, 4, 256] chunks 0-3
            dbt_hi = acd_sb.bitcast(BF16).rearrange(
                "p (c w) -> p c w", c=4)          # chunks 4-7
            aux_f = aux_sb.bitcast(F32)

            # ---- PSUM ----
            ps_kc0 = pp.tile([128, 512], F32, tag="pskc0", name="pskc0")
            ps_kc1 = pp.tile([128, 512], F32, tag="pskc1", name="pskc1")
            ps_db = pp.tile([128, 2 * B], F32, tag="psdb", name="psdb")
            trall = pp.tile([128, 4, 128], BF16, tag="trall", name="trall")
            trall2 = pp.tile([128, 4, 128], BF16, tag="trall2", name="trall2")
            ps_out_lo = pp.tile([128, 512], F32, tag="psoutl", name="psoutl")
            ps_out_hi = pp.tile([128, 512], F32, tag="psouth", name="psouth")

            # ================= constant generation =================
            # gpsimd: staging + iotas + cc m1 = j*s + t2
            t2c = pool.tile([128, 8], I32, tag="t2c", name="t2c")
            nc.gpsimd.tensor_copy(t2c[:], aux_sb[:, 0:8])     # waits aux DMA
            wz = pool.tile([128, 640], BF16, tag="wz", name="wz")
            memset_h = nc.gpsimd.memset(wz[:], 0.0)
            ji = pool.tile([128, 8, 64], I32, tag="ji", name="ji")
            nc.gpsimd.iota(ji[:], pattern=[[128, 8], [0, 64]],
                           base=0, channel_multiplier=1)
            si = pool.tile([128, 8, 64], I32, tag="si", name="si")
            nc.gpsimd.iota(si[:], pattern=[[0, 8], [1, 64]],
                           base=0, channel_multiplier=0)
            jg = pool.tile([128, 1024], I32, tag="jg", name="jg")
            nc.gpsimd.iota(jg[:], pattern=[[1, 1024]], base=0,
                           channel_multiplier=0)
            m0 = pool.tile([128, 8, 64], I32, tag="m0", name="m0")
            nc.gpsimd.tensor_tensor(m0[:], ji[:], si[:], op=MUL_)
            m1 = pool.tile([128, 8, 64], I32, tag="m1", name="m1")
            gp_last = nc.gpsimd.tensor_tensor(
                m1[:], m0[:], t2c[:].unsqueeze(2).broadcast_to([128, 8, 64]),
                op=ADD_)

            # DVE: staging, cc index ALU + casts first (unblocks ACT), then g3
            fgc = pool.tile([128, 1], I32, tag="fgc", name="fgc")
            nc.vector.tensor_copy(fgc[:], aux_sb[:, 8:9])     # waits aux DMA
            scl = pool.tile([128, 3], F32, tag="scl", name="scl")
            nc.vector.tensor_copy(scl[:], aux_f[:, 9:12])     # sa, sb, pi
            im0 = pool.tile([128, 8], BF16, tag="im0", name="im0")
            nc.vector.tensor_copy(im0[:], aux_f[:, 12:20])
            mca = pool.tile([128, 8, 64], I32, tag="mca", name="mca")
            nc.vector.tensor_scalar(mca[:], m1[:], 256, None, op0=ADD_)
            nc.vector.tensor_scalar(mca[:], mca[:], 1023, None, op0=AND_)
            msa = pool.tile([128, 8, 64], I32, tag="msa", name="msa")
            nc.vector.tensor_scalar(msa[:], m1[:], 512, None, op0=ADD_)
            nc.vector.tensor_scalar(msa[:], msa[:], 1023, None, op0=AND_)
            mcf = pool.tile([128, 8, 64], F32, tag="mcf", name="mcf")
            nc.vector.tensor_copy(mcf[:], mca[:])
            msf = pool.tile([128, 8, 64], F32, tag="msf", name="msf")
            nc.vector.tensor_copy(msf[:], msa[:])
            # g3 indices (jg staged onto DVE so m2 needs one self-wait)
            jgd = pool.tile([128, 1024], I32, tag="jgd", name="jgd")
            nc.vector.tensor_copy(jgd[:], jg[:])
            m2 = pool.tile([128, 1024], I32, tag="m2", name="m2")
            nc.vector.tensor_tensor(m2[:], jgd[:],
                                    fgc[:].broadcast_to([128, 1024]), op=MUL_)
            ga_i = pool.tile([128, 1024], I32, tag="gai", name="gai")
            nc.vector.tensor_scalar(ga_i[:], m2[:], 256, None, op0=ADD_)
            nc.vector.tensor_scalar(ga_i[:], ga_i[:], 1023, None, op0=AND_)
            gs_i = pool.tile([128, 1024], I32, tag="gsi", name="gsi")
            nc.vector.tensor_scalar(gs_i[:], m2[:], 512, None, op0=ADD_)
            nc.vector.tensor_scalar(gs_i[:], gs_i[:], 1023, None, op0=AND_)
            gaf = pool.tile([128, 1024], F32, tag="gaf", name="gaf")
            nc.vector.tensor_copy(gaf[:], ga_i[:])
            gsf = pool.tile([128, 1024], F32, tag="gsf", name="gsf")
            nc.vector.tensor_copy(gsf[:], gs_i[:])

            # ACT: four Sins (single Sin-table residency)
            cc_raw = pool.tile([128, 8, 128], BF16, tag="ccraw", name="ccraw")
            nc.scalar.activation(cc_raw[:, :, 0:64], mcf[:], SIN,
                                 bias=scl[:, 2:3], scale=S2PI)
            nc.scalar.activation(cc_raw[:, :, 64:128], msf[:], SIN,
                                 bias=scl[:, 2:3], scale=S2PI)
            g3a_raw = pool.tile([128, 1024], BF16, tag="g3ar", name="g3ar")
            nc.scalar.activation(g3a_raw[:], gaf[:], SIN,
                                 bias=scl[:, 2:3], scale=S2PI)
            g3b_raw = pool.tile([128, 1024], BF16, tag="g3br", name="g3br")
            act_g_last = nc.scalar.activation(g3b_raw[:], gsf[:], SIN,
                                              bias=scl[:, 2:3], scale=S2PI)

            # DVE: finalize cc (single producer for the PE wait) + g3 scales
            cc_t = pool.tile([128, 8, 128], BF16, tag="cct", name="cct")
            nc.vector.tensor_copy(
                cc_t[:].rearrange("p c s -> p (c s)"),
                cc_raw[:].rearrange("p c s -> p (c s)"))
            nc.vector.tensor_copy(cc_t[:, :, 64:65],
                                  im0[:].unsqueeze(2))
            g3a_v = pool.tile([128, 1024], BF16, tag="g3a", name="g3a")
            nc.vector.tensor_scalar(g3a_v[:], g3a_raw[:], scl[:, 0:1], None,
                                    op0=MUL_)
            g3b_v = pool.tile([128, 1024], BF16, tag="g3b", name="g3b")
            nc.vector.tensor_scalar(g3b_v[:], g3b_raw[:], scl[:, 1:2], None,
                                    op0=MUL_)

            # ================= main pipeline =================
            # PE warmup junk into ps_out (S4 start=True overwrites)
            for w in range(4):
                nc.tensor.matmul(ps_out_lo[:], wz[:, :128], wz[:, 128:640],
                                 start=True, stop=True)

            # S1 h0
            h_order = [0, 1, 4, 5, 2, 3, 6, 7]
            for i, c in enumerate(h_order):
                nc.tensor.matmul(ps_kc0[:], cc_t[:, c, :], ktv[(c, 0)],
                                 start=(i == 0), stop=(i == 7))
            # T1 h0 (kcT copy on DVE, transposes on PE, kc copy on DVE)
            kcT0 = pool.tile([128, 512], BF16, tag="kcT0", name="kcT0")
            nc.vector.tensor_copy(kcT0[:], ps_kc0[:])
            for c in range(4):
                nc.tensor.transpose(trall[:, c, :],
                                    kcT0[:, c * 128:(c + 1) * 128], id_v)
            kc_lo = pool.tile([128, 4, 128], BF16, tag="kclo", name="kclo")
            nc.vector.tensor_copy(
                kc_lo[:].rearrange("p c s -> p (c s)"),
                trall[:].rearrange("p c s -> p (c s)"))

            # S1 h1 first pair + S2 partial 0 + S1 h1 rest
            for i, c in enumerate(h_order[:2]):
                nc.tensor.matmul(ps_kc1[:], cc_t[:, c, :], ktv[(c, 1)],
                                 start=(i == 0), stop=False)
            for c in range(4):
                nc.tensor.matmul(ps_db[:], kc_lo[:, c, :], dbt_lo[:, c, :],
                                 start=(c == 0), stop=False)
            for i, c in enumerate(h_order[2:]):
                nc.tensor.matmul(ps_kc1[:], cc_t[:, c, :], ktv[(c, 1)],
                                 start=False, stop=(i == 5))

            # T1 h1
            kcT1 = pool.tile([128, 512], BF16, tag="kcT1", name="kcT1")
            nc.vector.tensor_copy(kcT1[:], ps_kc1[:])
            for c in range(4):
                nc.tensor.transpose(trall2[:, c, :],
                                    kcT1[:, c * 128:(c + 1) * 128], id_v)
            kc_hi = pool.tile([128, 4, 128], BF16, tag="kchi", name="kchi")
            nc.vector.tensor_copy(
                kc_hi[:].rearrange("p c s -> p (c s)"),
                trall2[:].rearrange("p c s -> p (c s)"))

            # S2 partial 1
            for c in range(4):
                nc.tensor.matmul(ps_db[:], kc_hi[:, c, :], dbt_hi[:, c, :],
                                 start=False, stop=(c == 3))

            # PW in [s, b] layout
            db_sb = pool.tile([128, 2 * B], BF16, tag="db", name="db")
            nc.vector.tensor_copy(db_sb[:], ps_db[:])
            dbsw = pool.tile([128, B], BF16, tag="dbsw", name="dbsw")
            nc.vector.tensor_copy(dbsw[0:64, :], db_sb[64:128, B:2 * B])
            nc.vector.tensor_copy(dbsw[64:128, :], db_sb[0:64, B:2 * B])
            ptA = pool.tile([128, B], BF16, tag="ptA", name="ptA")
            ptC2 = pool.tile([128, B], BF16, tag="ptC2", name="ptC2")
            nc.vector.tensor_mul(ptA[:], db_sb[:, 0:B], db_sb[:, B:2 * B])
            nc.vector.tensor_mul(ptC2[:], db_sb[:, 0:B], dbsw[:])

            # S4 per bank + cast + store
            out_lo = pool.tile([128, 512], BF16, tag="outlo", name="outlo")
            out_hi = pool.tile([128, 512], BF16, tag="outhi", name="outhi")
            stores = []
            nc.tensor.matmul(ps_out_lo[:], ptA[:], g3a_v[:, 0:512],
                             start=True, stop=False)
            nc.tensor.matmul(ps_out_lo[:], ptC2[:], g3b_v[:, 0:512],
                             start=False, stop=True)
            cp_lo = nc.scalar.copy(out_lo[:], ps_out_lo[:])
            stores.append(nc.sync.dma_start(out[:, :256],
                                            out_lo.bitcast(F32)[:, :]))
            nc.tensor.matmul(ps_out_hi[:], ptA[:], g3a_v[:, 512:1024],
                             start=True, stop=False)
            last_mm = nc.tensor.matmul(ps_out_hi[:], ptC2[:],
                                       g3b_v[:, 512:1024],
                                       start=False, stop=True)
            cp_hi = nc.vector.tensor_copy(out_hi[:], ps_out_hi[:])
            stores.append(nc.scalar.dma_start(out[:, 256:],
                                              out_hi.bitcast(F32)[:, :]))

            # tail: absorb every outstanding tick into SP's clock
            prev = None
            for dep in [*in_dmas, memset_h, gp_last, act_g_last, *stores,
                        last_mm, cp_lo, cp_hi]:
                dr = nc.sync.drain(fusable=False)
                add_dep_helper(dr.ins, dep.ins, sync=True,
                               reason="tail: absorb tick into SP clock")
                if prev is not None:
                    add_dep_helper(dr.ins, prev.ins, sync=False,
                                   reason="tail: keep drain chain ordered")
                prev = dr

    return nc


def _bf16_pack(a):
    """float32 (P, W) -> bf16 packed two-per-word as float32 (P, W//2)."""
    bf = np.ascontiguousarray(np.asarray(a, np.float32).astype(ml_dtypes.bfloat16))
    return bf.view(np.uint8).reshape(bf.shape[0], -1).view(np.float32)


def _partition_pack(a):
    """(n*128, W) -> (128, n*W): row p = concat of chunk rows p."""
    r, w = a.shape
    n = r // 128
    return np.ascontiguousarray(
        a.reshape(n, 128, w).transpose(1, 0, 2).reshape(128, n * w))


def _aux_for_core(core):
    p = np.arange(128)
    c = np.arange(8)
    j = c[None, :] * 128 + p[:, None]                 # [128, 8]
    auxm = np.zeros((128, AUX_W), np.int32)
    auxm[:, 0:8] = (64 * core * j) % 1024
    fg = 64 * core + (p % 64)
    sa = np.where(p < 64, 4.0 / N, -4.0 / N).astype(np.float32)
    # raw g3b value is sin(th+pi) = -sin(th); C = -w sin(th) -> scale +w
    sb = np.full(128, 4.0 / N, np.float32)
    # cc im slot-0 override: normal value is -sin(2pi j (64 core)/1024);
    # core 0 carries the Nyquist cos(pi j) = (-1)^j column instead.
    if core == 0:
        im0 = np.cos(np.pi * j).astype(np.float32)
        fg = fg.copy()
        fg[64] = 512                 # Bm row 0 -> (2/N) cos(pi j)
        sa = sa.copy()
        sa[0] = 2.0 / N              # A row 0 -> 2/N constant (cos(0)=1)
        sa[64] = 2.0 / N             # Bm row 0 sign+scale
        sb = sb.copy()
        sb[0] = 0.0                  # C rows 0 and dup are zero
        sb[64] = 0.0
    else:
        im0 = -np.sin(2.0 * np.pi * j * (64 * core) / N).astype(np.float32)
    fv = np.zeros((128, AUX_W), np.float32)
    fv[:, AUX_SA] = sa
    fv[:, AUX_SB] = sb
    fv[:, AUX_PI] = np.pi
    fv[:, AUX_IM0:AUX_IM0 + 8] = im0
    auxm[:, AUX_FG] = fg
    outm = auxm.copy()
    outm[:, AUX_SA:] = fv[:, AUX_SA:].view(np.int32)
    return np.ascontiguousarray(outm)


def kernel(des, body, kernel):
    global LAST_RESULT
    K = np.asarray(kernel, dtype=np.float32)
    des = np.asarray(des, dtype=np.float32)
    body = np.asarray(body, dtype=np.float32)

    ktb = K.T.astype(ml_dtypes.bfloat16)                # (1024 j, 1024 k)
    def ktpk(c, h):
        blk = np.ascontiguousarray(
            ktb[c * 128:(c + 1) * 128, h * 512:(h + 1) * 512], np.float32)
        return _bf16_pack(blk)                          # (128, 256) words

    id_pk = _bf16_pack(np.eye(128, dtype=np.float32))   # (128, 64) words
    dbt_np = np.concatenate([des.T, body.T], axis=1)    # (1024, 256)
    dbt_pk = _partition_pack(_bf16_pack(dbt_np))        # (128, 1024) words

    sp1v = np.ascontiguousarray(np.concatenate(
        [id_pk] + [ktpk(c, 0) for c in range(4)], axis=1))
    spdv = np.ascontiguousarray(dbt_pk[:, 0:512])
    sp2v = np.ascontiguousarray(np.concatenate(
        [ktpk(c, 1) for c in range(4)], axis=1))
    ac1v = np.ascontiguousarray(np.concatenate(
        [ktpk(4 + c, 0) for c in range(4)], axis=1))
    acdv = np.ascontiguousarray(dbt_pk[:, 512:1024])
    ac2v = np.ascontiguousarray(np.concatenate(
        [ktpk(4 + c, 1) for c in range(4)], axis=1))

    in_maps = []
    for core in range(N_CORES):
        in_maps.append({
            "sp1": sp1v, "spd": spdv, "sp2": sp2v,
            "ac1": ac1v, "acd": acdv, "ac2": ac2v,
            "aux": _aux_for_core(core),
        })

    if "nc" not in _nc_cache:
        _nc_cache["nc"] = _build_nc()
    nc = _nc_cache["nc"]

    res = run_bass_kernel_spmd(nc, in_maps, list(range(N_CORES)))
    LAST_RESULT = res
    out = np.zeros((B, N), dtype=np.float32)
    for r in res.results:
        w = np.ascontiguousarray(np.asarray(r["out"], np.float32))
        bf = w.view(np.uint8).reshape(B, -1).view(ml_dtypes.bfloat16)
        out += bf.astype(np.float32)
    return out


# revision 13
# speedup vs baseline: 1.0305x; 1.0224x over previous
r"""Circulant layer kernel for Trainium2 (8 NeuronCores) — v5.

Math (same as v2): reference computes mv1 + mv2 = 2 * circconv(d, b)
with d = des @ K, b = body @ K.  Real-input half-spectrum DFT: cores
0..7 own freqs f = 64c..64c+63; Nyquist f=512 rides core 0's slot-0
imaginary column with the generalized 3-product inverse (G3).

v5 queue layout (HWDGE has 8 semaphore slots total; SWDGE measured at
only ~65-100GB/s so it carries the small/late constants):
  SP:  [id | kt h0 c0-3] [dbt c0-3] [kt h1 c0-3]      (3 DMAs)
  ACT: [kt h0 c4-7] [dbt c4-7] [kt h1 c4-7]           (3 DMAs)
  GP (SWDGE): [cc] [g3a] [g3b]   (cc needed ~14us, g3 only at S4 ~23us)
  stores: SP lo + ACT hi                               (2 DMAs) = 8 HWDGE

v3 structural changes vs v2 (40.9us measured -> 38.1):
  * K^T streams on BOTH hardware DMA queues (SP + ACT), split by
    j-chunk pairs and k-halves; cc/dbt ride the gpsimd SWDGE queue.
    (v2 put all of kt on one queue at ~190GB/s — the single-queue
    stream, not PE, set the critical path.)
  * k-half phasing: S1 (KC^T = CC^T K^T) accumulates k-half 0 in PSUM
    bank 0 and k-half 1 in bank 1, so T1/S2 for half 0 run while
    half 1 is still streaming in.
  * The pointwise spectral products are computed directly in [s, b]
    layout from S2's output (DVE ops with partition-base-shifted
    operands — verified on HW), eliminating T2, T3 and their staging
    copies entirely:
      ptA[p, b]        = db[p, b] * db[p, B+b]          (p = 0..127)
      ptC2[p, b]       = db[p, b] * db[(p+64)%128, B+b] (two half ops)
    ptA/ptC2 feed S4 as stationaries with G3a / duplicated-C moving.
  * S4 + cast + store issue per 512-col PSUM bank as soon as ready.

Fixed costs measured by probe: ~8.3us preamble, ~2us DMA issue->land,
~2us store issue->tick, ~8.3us after last store tick.
"""

import numpy as np
import ml_dtypes

import concourse.bass as bass
import concourse.mybir as mybir
import concourse.tile as tile
from concourse.bass_utils import run_bass_kernel_spmd
from concourse.tile_rust import add_dep_helper

B = 128        # batch
D_IN = 1024    # input feature dim (contraction k)
N = 1024       # output feature dim (conv length j)
N_CORES = 8
FPC = 64       # complex frequency slots per core
S = 2 * FPC    # 128 freq columns per core: [0:64]=re(cos), [64:128]=im(-sin)

F32 = mybir.dt.float32
BF16 = mybir.dt.bfloat16

LAST_RESULT = None
_nc_cache = {}


def _build_nc():
    nc = bass.Bass(target_bir_lowering=True)

    # --- DRAM params (bf16 packed two-per-f32-word) ---
    sp1 = nc.declare_dram_parameter("sp1", [128, 64 + 1024], F32, False)
    spd = nc.declare_dram_parameter("spd", [128, 512], F32, False)
    sp2 = nc.declare_dram_parameter("sp2", [128, 1024], F32, False)
    ac1 = nc.declare_dram_parameter("ac1", [128, 1024], F32, False)
    acd = nc.declare_dram_parameter("acd", [128, 512], F32, False)
    ac2 = nc.declare_dram_parameter("ac2", [128, 1024], F32, False)
    cc = nc.declare_dram_parameter("cc", [128, 512], F32, False)
    g3a = nc.declare_dram_parameter("g3a", [128, 512], F32, False)
    g3b = nc.declare_dram_parameter("g3b", [128, 512], F32, False)
    out = nc.declare_dram_parameter("out", [B, N // 2], F32, isOutput=True)

    with tile.TileContext(nc) as tc:
        with (
            tc.tile_pool(name="main", bufs=1) as pool,
            tc.tile_pool(name="psum", bufs=1, space="PSUM") as pp,
        ):
            # ---- input DMAs, phase-ordered per queue ----
            sp1_sb = pool.tile([128, 64 + 1024], F32, tag="sp1", name="sp1")
            spd_sb = pool.tile([128, 512], F32, tag="spd", name="spd")
            sp2_sb = pool.tile([128, 1024], F32, tag="sp2", name="sp2")
            ac1_sb = pool.tile([128, 1024], F32, tag="ac1", name="ac1")
            acd_sb = pool.tile([128, 512], F32, tag="acd", name="acd")
            ac2_sb = pool.tile([128, 1024], F32, tag="ac2", name="ac2")
            cc_sb = pool.tile([128, 512], F32, tag="cc", name="cc")
            g3a_sb = pool.tile([128, 512], F32, tag="g3a", name="g3a")
            g3b_sb = pool.tile([128, 512], F32, tag="g3b", name="g3b")

            in_dmas = []
            in_dmas.append(nc.sync.dma_start(sp1_sb[:], sp1[:, :]))
            in_dmas.append(nc.sync.dma_start(spd_sb[:], spd[:, :]))
            in_dmas.append(nc.sync.dma_start(sp2_sb[:], sp2[:, :]))
            in_dmas.append(nc.scalar.dma_start(ac1_sb[:], ac1[:, :]))
            in_dmas.append(nc.scalar.dma_start(acd_sb[:], acd[:, :]))
            in_dmas.append(nc.scalar.dma_start(ac2_sb[:], ac2[:, :]))
            in_dmas.append(nc.gpsimd.dma_start(cc_sb[:], cc[:, :]))
            in_dmas.append(nc.gpsimd.dma_start(g3a_sb[:], g3a[:, :]))
            in_dmas.append(nc.gpsimd.dma_start(g3b_sb[:], g3b[:, :]))

            # bf16 views
            id_v = sp1_sb.bitcast(BF16)[:, 0:128]
            # kt[c][h] -> [128, 512] bf16 view
            sp1v = sp1_sb.bitcast(BF16)
            sp2v = sp2_sb.bitcast(BF16)
            ac1v = ac1_sb.bitcast(BF16)
            ac2v = ac2_sb.bitcast(BF16)
            ktv = {}
            for c in range(4):
                ktv[(c, 0)] = sp1v[:, 128 + c * 512:128 + (c + 1) * 512]
                ktv[(c, 1)] = sp2v[:, c * 512:(c + 1) * 512]
                ktv[(4 + c, 0)] = ac1v[:, c * 512:(c + 1) * 512]
                ktv[(4 + c, 1)] = ac2v[:, c * 512:(c + 1) * 512]
            g3a_v = g3a_sb.bitcast(BF16)          # [128, 1024]
            g3b_v = g3b_sb.bitcast(BF16)          # [128, 1024]
            cc_v = cc_sb.bitcast(BF16).rearrange(
                "p (c s) -> p c s", c=8)          # [128, 8, 128]
            dbt_lo = spd_sb.bitcast(BF16).rearrange(
                "p (c w) -> p c w", c=4)          # [128, 4, 256] k-chunks 0-3
            dbt_hi = acd_sb.bitcast(BF16).rearrange(
                "p (c w) -> p c w", c=4)          # k-chunks 4-7

            # ---- PSUM layout ----
            ps_kc0 = pp.tile([128, 512], F32, tag="pskc0", name="pskc0")
            ps_kc1 = pp.tile([128, 512], F32, tag="pskc1", name="pskc1")
            ps_db = pp.tile([128, 2 * B], F32, tag="psdb", name="psdb")
            trall = pp.tile([128, 4, 128], BF16, tag="trall", name="trall")
            trall2 = pp.tile([128, 4, 128], BF16, tag="trall2", name="trall2")
            ps_out_lo = pp.tile([128, 512], F32, tag="psoutl", name="psoutl")
            ps_out_hi = pp.tile([128, 512], F32, tag="psouth", name="psouth")
            ps_junk = pp.tile([128, 512], F32, tag="psjunk", name="psjunk")

            # ---- PE warmup: junk matmuls into a dedicated junk bank ----
            wz = pool.tile([128, 640], BF16, tag="wz", name="wz")
            memset_h = nc.gpsimd.memset(wz[:], 0.0)
            for w in range(6):
                nc.tensor.matmul(ps_junk[:], wz[:, :128], wz[:, 128:640],
                                 start=True, stop=True)

            # ---- S1 phase 0: ps_kc0[s, k0:512] = sum_j cc[j,s]^T kt[j, h0] ----
            # ACT's kt h0 DMA lands before SP's (which also carries id+cc
            # riders); consume ACT chunks first
            h0_order = [4, 5, 6, 7, 0, 1, 2, 3]
            for i, c in enumerate(h0_order):
                nc.tensor.matmul(ps_kc0[:], cc_v[:, c, :], ktv[(c, 0)],
                                 start=(i == 0), stop=(i == 7))

            # ---- T1 phase 0: transpose KC^T[:, 0:512] -> kc chunks 0..3 ----
            kcT0 = pool.tile([128, 512], BF16, tag="kcT0", name="kcT0")
            nc.scalar.copy(kcT0[:], ps_kc0[:])
            for w in range(2):
                nc.tensor.matmul(ps_junk[:, 0:256], wz[:, :128], wz[:, 128:384],
                                 start=True, stop=True)
            for c in range(4):
                nc.tensor.transpose(trall[:, c, :],
                                    kcT0[:, c * 128:(c + 1) * 128], id_v)
            kc_lo = pool.tile([128, 4, 128], BF16, tag="kclo", name="kclo")
            nc.vector.tensor_copy(
                kc_lo[:].rearrange("p c s -> p (c s)"),
                trall[:].rearrange("p c s -> p (c s)"))

            # ---- S2 partial 0: ps_db += kc[k0 chunks] @ dbt ----
            for c in range(4):
                nc.tensor.matmul(ps_db[:], kc_lo[:, c, :], dbt_lo[:, c, :],
                                 start=(c == 0), stop=False)

            # clock-hold junk while kt h1 streams in
            for w in range(4):
                nc.tensor.matmul(ps_junk[:, 0:256], wz[:, :128], wz[:, 128:384],
                                 start=True, stop=True)

            # ---- S1 phase 1 ----
            h1_order = [4, 5, 6, 7, 0, 1, 2, 3]
            for i, c in enumerate(h1_order):
                nc.tensor.matmul(ps_kc1[:], cc_v[:, c, :], ktv[(c, 1)],
                                 start=(i == 0), stop=(i == 7))

            # ---- T1 phase 1 ----
            kcT1 = pool.tile([128, 512], BF16, tag="kcT1", name="kcT1")
            nc.scalar.copy(kcT1[:], ps_kc1[:])
            for c in range(4):
                nc.tensor.transpose(trall2[:, c, :],
                                    kcT1[:, c * 128:(c + 1) * 128], id_v)
            # separate PSUM tile (trall2): a second read of the same PSUM
            # tile needs two sync waits, which compute instrs can't encode
            kc_hi = pool.tile([128, 4, 128], BF16, tag="kchi", name="kchi")
            nc.vector.tensor_copy(
                kc_hi[:].rearrange("p c s -> p (c s)"),
                trall2[:].rearrange("p c s -> p (c s)"))

            # ---- S2 partial 1 ----
            for c in range(4):
                nc.tensor.matmul(ps_db[:], kc_hi[:, c, :],
                                 dbt_hi[:, c, :],
                                 start=False, stop=(c == 3))

            # ---- PW in [s, b] layout (no transposes) ----
            db_sb = pool.tile([128, 2 * B], BF16, tag="db", name="db")
            nc.vector.tensor_copy(db_sb[:], ps_db[:])
            # partition-swapped copy of the B half (tensor_tensor requires
            # same start partition on all APs; tensor_copy does not)
            dbsw = pool.tile([128, B], BF16, tag="dbsw", name="dbsw")
            nc.vector.tensor_copy(dbsw[0:64, :], db_sb[64:128, B:2 * B])
            nc.vector.tensor_copy(dbsw[64:128, :], db_sb[0:64, B:2 * B])
            ptA = pool.tile([128, B], BF16, tag="ptA", name="ptA")
            ptC2 = pool.tile([128, B], BF16, tag="ptC2", name="ptC2")
            nc.vector.tensor_mul(ptA[:], db_sb[:, 0:B], db_sb[:, B:2 * B])
            nc.vector.tensor_mul(ptC2[:], db_sb[:, 0:B], dbsw[:])

            # ---- S4 per bank: out[b, j] = ptA^T G3a + ptC2^T G3b2 ----
            out_lo = pool.tile([128, 512], BF16, tag="outlo", name="outlo")
            out_hi = pool.tile([128, 512], BF16, tag="outhi", name="outhi")
            stores = []
            nc.tensor.matmul(ps_out_lo[:], ptA[:], g3a_v[:, 0:512],
                             start=True, stop=False)
            nc.tensor.matmul(ps_out_lo[:], ptC2[:], g3b_v[:, 0:512],
                             start=False, stop=True)
            cp_lo = nc.scalar.copy(out_lo[:], ps_out_lo[:])
            stores.append(nc.sync.dma_start(out[:, :256],
                                            out_lo.bitcast(F32)[:, :]))
            nc.tensor.matmul(ps_out_hi[:], ptA[:], g3a_v[:, 512:1024],
                             start=True, stop=False)
            last_mm = nc.tensor.matmul(ps_out_hi[:], ptC2[:],
                                       g3b_v[:, 512:1024],
                                       start=False, stop=True)
            cp_hi = nc.vector.tensor_copy(out_hi[:], ps_out_hi[:])
            stores.append(nc.scalar.dma_start(out[:, 256:],
                                              out_hi.bitcast(F32)[:, :]))

            # ---- tail: absorb every outstanding tick into SP's clock ----
            prev = None
            for dep in [*in_dmas, memset_h, *stores, last_mm, cp_lo, cp_hi]:
                dr = nc.sync.drain(fusable=False)
                add_dep_helper(dr.ins, dep.ins, sync=True,
                               reason="tail: absorb tick into SP clock")
                if prev is not None:
                    add_dep_helper(dr.ins, prev.ins, sync=False,
                                   reason="tail: keep drain chain ordered")
                prev = dr

    return nc


def _bf16_pack(a):
    """float32 (P, W) -> bf16 packed two-per-word as float32 (P, W//2)."""
    bf = np.ascontiguousarray(np.asarray(a, np.float32).astype(ml_dtypes.bfloat16))
    return bf.view(np.uint8).reshape(bf.shape[0], -1).view(np.float32)


def _partition_pack(a):
    """(n*128, W) -> (128, n*W): row p = concat of chunk rows p."""
    r, w = a.shape
    n = r // 128
    return np.ascontiguousarray(
        a.reshape(n, 128, w).transpose(1, 0, 2).reshape(128, n * w))


def _constants():
    """Per-core CC [N, S], G3a [128, N], G3b2 [128, N] float32."""
    j = np.arange(N, dtype=np.float64)
    alt = np.cos(np.pi * j)                     # (-1)^j
    ccs, g3as, g3bs = [], [], []
    for c in range(N_CORES):
        f = np.arange(c * FPC, (c + 1) * FPC, dtype=np.float64)
        ang = 2.0 * np.pi * np.outer(j, f) / N             # (j, t)
        cc_re = np.cos(ang)
        cc_im = -np.sin(ang)
        angT = ang.T                                        # (t, j)
        w = 4.0 / N
        A = w * np.cos(angT)                                # m0 rows
        Bm = -w * np.cos(angT)                              # m1 rows
        C = -w * np.sin(angT)                               # mC rows
        if c == 0:
            cc_im[:, 0] = alt                               # f=512 cos column
            A[0, :] = 2.0 / N                               # m0 = D0*B0
            Bm[0, :] = (2.0 / N) * alt                      # m1 = D512*B512
            C[0, :] = 0.0
        cc_full = np.concatenate([cc_re, cc_im], axis=1)    # (N, 128)
        ccs.append(np.ascontiguousarray(cc_full, np.float32))
        g3as.append(np.ascontiguousarray(
            np.concatenate([A, Bm], axis=0), np.float32))       # (128, N)
        g3bs.append(np.ascontiguousarray(
            np.concatenate([C, C], axis=0), np.float32))        # (128, N)
    return ccs, g3as, g3bs


def kernel(des, body, kernel):
    global LAST_RESULT
    K = np.asarray(kernel, dtype=np.float32)
    des = np.asarray(des, dtype=np.float32)
    body = np.asarray(body, dtype=np.float32)

    # K^T as bf16 blocks: block (c, h) = K^T[c*128:(c+1)*128, h*512:(h+1)*512]
    ktb = K.T.astype(ml_dtypes.bfloat16)                # (1024 j, 1024 k)
    def ktpk(c, h):
        blk = np.ascontiguousarray(
            ktb[c * 128:(c + 1) * 128, h * 512:(h + 1) * 512], np.float32)
        return _bf16_pack(blk)                          # (128, 256) words

    id_pk = _bf16_pack(np.eye(128, dtype=np.float32))   # (128, 64) words
    dbt_np = np.concatenate([des.T, body.T], axis=1)    # (1024, 256)
    dbt_pk = _partition_pack(_bf16_pack(dbt_np))        # (128, 1024) words

    ccs, g3as, g3bs = _constants()
    in_maps = []
    for c in range(N_CORES):
        cc_pk = _partition_pack(_bf16_pack(ccs[c]))     # (128, 512) words
        m = {
            "sp1": np.ascontiguousarray(np.concatenate(
                [id_pk, ktpk(0, 0), ktpk(1, 0), ktpk(2, 0), ktpk(3, 0)],
                axis=1)),
            "spd": np.ascontiguousarray(dbt_pk[:, 0:512]),
            "sp2": np.ascontiguousarray(np.concatenate(
                [ktpk(0, 1), ktpk(1, 1), ktpk(2, 1), ktpk(3, 1)], axis=1)),
            "ac1": np.ascontiguousarray(np.concatenate(
                [ktpk(4, 0), ktpk(5, 0), ktpk(6, 0), ktpk(7, 0)], axis=1)),
            "acd": np.ascontiguousarray(dbt_pk[:, 512:1024]),
            "ac2": np.ascontiguousarray(np.concatenate(
                [ktpk(4, 1), ktpk(5, 1), ktpk(6, 1), ktpk(7, 1)], axis=1)),
            "cc": cc_pk,
            "g3a": np.ascontiguousarray(_bf16_pack(g3as[c])),
            "g3b": np.ascontiguousarray(_bf16_pack(g3bs[c])),
        }
        in_maps.append(m)

    if "nc" not in _nc_cache:
        _nc_cache["nc"] = _build_nc()
    nc = _nc_cache["nc"]

    res = run_bass_kernel_spmd(nc, in_maps, list(range(N_CORES)))
    LAST_RESULT = res
    out = np.zeros((B, N), dtype=np.float32)
    for r in res.results:
        w = np.ascontiguousarray(np.asarray(r["out"], np.float32))
        bf = w.view(np.uint8).reshape(B, -1).view(ml_dtypes.bfloat16)
        out += bf.astype(np.float32)
    return out


# revision 15
# speedup vs baseline: 1.0736x; 1.0419x over previous
r"""Circulant layer kernel for Trainium2 (8 NeuronCores) — v6.

Math: reference computes mv1 + mv2 = 2 * circconv(d, b) with
d = des @ K, b = body @ K, via a real-input half-spectrum DFT.
Cores 0..7 own freqs f = 64c..64c+63; the Nyquist f=512 rides core 0's
slot-0 imaginary column, with the generalized 3-product (G3) inverse
folded into per-partition scales and a host-supplied override column.

Measured machine facts this version is built around:
  * Fixed overhead: ~8.5us preamble, ~10.2us after the last store
    issue (DMA proxy quiesce + final rendezvous).  Only the span
    first-DMA-issue -> last-store-issue is controllable.
  * Input DMA: ~65-110GB/s per queue early (8 cores contending),
    faster later; SWDGE is slower still.  Fewer, larger HWDGE DMAs
    beat many small ones.  8 HWDGE semaphore slots total.
  * PE: hot back-to-back matmuls hit 1 col/cycle but the clock ramps
    only under sustained work (cold mms are ~1.7x slower); junk
    matmuls into a scratch PSUM bank keep it warm.
  * Engine ALU: DVE ~0.5ns/elem f32/i32 (2x for 16-bit), gpsimd
    ~10-20ns/elem — only iotas/memset live there.  ACT Sin table
    covers (-pi, pi]: sin(2pi m/1024) == Sin(pi - m*2pi/1024).

Structure:
  * kt (K^T bf16, 2MB) streams as 4 big HWDGE DMAs, k-half phased:
    SP [id|kt h0 c0-3] [kt h1 c0-3], ACT [kt h0 c4-7] [kt h1 c4-7].
    S1 accumulates each k-half in its own PSUM bank; T1 (PE transpose)
    and S2 for half 0 run while half 1 still streams.
  * dbt (des^T|body^T, 512KB) + a 10KB aux ride the SWDGE queue.
  * CC and G3a/G3b trig tables are GENERATED on device (saves 768KB
    of stream): gpsimd iotas -> int16 index ALU on DVE (wraparound is
    exact mod 2^16, and &1023 after) -> ACT Sin -> DVE scales.
  * Pointwise spectral products in [s, b] layout straight out of S2
    (partition-base-shifted copies; no T2/T3 transposes).
  * S4 + cast + store issue per 512-col PSUM bank.

One-sync-wait discipline: compute instrs encode exactly one wait, so
DMA-landed operands are staged through a same-engine copy and
cross-engine handoffs put the producer wait on the first consumer.
"""

import numpy as np
import ml_dtypes

import concourse.bass as bass
import concourse.mybir as mybir
import concourse.tile as tile
from concourse.bass_utils import run_bass_kernel_spmd
from concourse.tile_rust import add_dep_helper

B = 128        # batch
D_IN = 1024    # input feature dim (contraction k)
N = 1024       # output feature dim (conv length j)
N_CORES = 8
FPC = 64       # complex frequency slots per core

F32 = mybir.dt.float32
I32 = mybir.dt.int32
I16 = mybir.dt.int16
BF16 = mybir.dt.bfloat16
SIN = mybir.ActivationFunctionType.Sin
ADD_ = mybir.AluOpType.add
AND_ = mybir.AluOpType.bitwise_and
MUL_ = mybir.AluOpType.mult
S2PI = float(-2.0 * np.pi / 1024.0)   # Sin(pi - m*2pi/1024) = sin(2pi m/1024)

# aux layout [128, 24] f32-declared:
#   i32 view cols [0:8) t2 = (64*core*j)%1024; col 8 fg (G3 row freq)
#   f32 cols 9,10,11 = sa, sb, pi;  cols [12:20) cc im-slot-0 override
AUXW = 24

LAST_RESULT = None
_nc_cache = {}


def _build_nc():
    nc = bass.Bass(target_bir_lowering=True)

    sp1 = nc.declare_dram_parameter("sp1", [128, 64 + 1024], F32, False)
    sp2 = nc.declare_dram_parameter("sp2", [128, 1024], F32, False)
    ac1 = nc.declare_dram_parameter("ac1", [128, 1024], F32, False)
    ac2 = nc.declare_dram_parameter("ac2", [128, 1024], F32, False)
    aux = nc.declare_dram_parameter("aux", [128, AUXW], F32, False)
    dbt = nc.declare_dram_parameter("dbt", [128, 1024], F32, False)
    out = nc.declare_dram_parameter("out", [B, N // 2], F32, isOutput=True)

    with tile.TileContext(nc) as tc:
        with (
            tc.tile_pool(name="main", bufs=1) as pool,
            tc.tile_pool(name="psum", bufs=1, space="PSUM") as pp,
        ):
            sp1_sb = pool.tile([128, 64 + 1024], F32, tag="sp1", name="sp1")
            sp2_sb = pool.tile([128, 1024], F32, tag="sp2", name="sp2")
            ac1_sb = pool.tile([128, 1024], F32, tag="ac1", name="ac1")
            ac2_sb = pool.tile([128, 1024], F32, tag="ac2", name="ac2")
            aux_sb = pool.tile([128, AUXW], F32, tag="aux", name="aux")
            dbt_sb = pool.tile([128, 1024], F32, tag="dbt", name="dbt")

            in_dmas = []
            in_dmas.append(nc.sync.dma_start(aux_sb[:], aux[:, :]))
            in_dmas.append(nc.sync.dma_start(sp1_sb[:], sp1[:, :]))
            in_dmas.append(nc.sync.dma_start(sp2_sb[:], sp2[:, :]))
            in_dmas.append(nc.scalar.dma_start(ac1_sb[:], ac1[:, :]))
            in_dmas.append(nc.scalar.dma_start(ac2_sb[:], ac2[:, :]))
            in_dmas.append(nc.gpsimd.dma_start(dbt_sb[:], dbt[:, :]))

            # views
            id_v = sp1_sb.bitcast(BF16)[:, 0:128]
            ktv = {}
            for c in range(4):
                ktv[(c, 0)] = sp1_sb.bitcast(BF16)[:, 128 + c * 512:
                                                   128 + (c + 1) * 512]
                ktv[(c, 1)] = sp2_sb.bitcast(BF16)[:, c * 512:(c + 1) * 512]
                ktv[(4 + c, 0)] = ac1_sb.bitcast(BF16)[:, c * 512:(c + 1) * 512]
                ktv[(4 + c, 1)] = ac2_sb.bitcast(BF16)[:, c * 512:(c + 1) * 512]
            dbt_v = dbt_sb.bitcast(BF16).rearrange(
                "p (c w) -> p c w", c=8)          # [128, 8, 256]
            aux_i = aux_sb.bitcast(I32)           # [128, 24]

            # PSUM
            ps_kc0 = pp.tile([128, 512], F32, tag="pskc0", name="pskc0")
            ps_kc1 = pp.tile([128, 512], F32, tag="pskc1", name="pskc1")
            ps_db = pp.tile([128, 2 * B], F32, tag="psdb", name="psdb")
            trall = pp.tile([128, 4, 128], BF16, tag="trall", name="trall")
            trall2 = pp.tile([128, 4, 128], BF16, tag="trall2", name="trall2")
            ps_out_lo = pp.tile([128, 512], F32, tag="psoutl", name="psoutl")
            ps_out_hi = pp.tile([128, 512], F32, tag="psouth", name="psouth")
            ps_junk = pp.tile([128, 512], F32, tag="psjunk", name="psjunk")

            # ============ constant generation ============
            # gpsimd: memset + int16 iotas only
            ji = pool.tile([128, 8, 64], I32, tag="ji", name="ji")
            nc.gpsimd.iota(ji[:], pattern=[[128, 8], [0, 64]],
                           base=0, channel_multiplier=1)
            si = pool.tile([128, 8, 64], I32, tag="si", name="si")
            nc.gpsimd.iota(si[:], pattern=[[0, 8], [1, 64]],
                           base=0, channel_multiplier=0)
            wz = pool.tile([128, 640], BF16, tag="wz", name="wz")
            memset_h = nc.gpsimd.memset(wz[:], 0.0)
            jgi = pool.tile([128, 1024], I32, tag="jgi", name="jgi")
            gp_last = nc.gpsimd.iota(jgi[:], pattern=[[1, 1024]], base=0,
                                     channel_multiplier=0)

            # DVE: stage aux (waits aux DMA), then cc chain, then g3 chain
            t2c = pool.tile([128, 8], I32, tag="t2c", name="t2c")
            nc.vector.tensor_copy(t2c[:], aux_i[:, 0:8])
            fgc = pool.tile([128, 1], I32, tag="fgc", name="fgc")
            nc.vector.tensor_copy(fgc[:], aux_i[:, 8:9])
            scl = pool.tile([128, 3], F32, tag="scl", name="scl")
            nc.vector.tensor_copy(scl[:], aux_sb[:, 9:12])
            im0 = pool.tile([128, 8], BF16, tag="im0", name="im0")
            nc.vector.tensor_copy(im0[:], aux_sb[:, 12:20])
            # cc: m1 = j*s + t2  (int32)
            m0 = pool.tile([128, 8, 64], I32, tag="m0", name="m0")
            nc.vector.tensor_tensor(m0[:], ji[:], si[:], op=MUL_)  # waits GP
            m1 = pool.tile([128, 8, 64], I32, tag="m1", name="m1")
            nc.vector.tensor_tensor(
                m1[:], m0[:], t2c[:].unsqueeze(2).broadcast_to([128, 8, 64]),
                op=ADD_)
            mca = pool.tile([128, 8, 64], I32, tag="mca", name="mca")
            nc.vector.tensor_scalar(mca[:], m1[:], 256, None, op0=ADD_)
            nc.vector.tensor_scalar(mca[:], mca[:], 1023, None, op0=AND_)
            msa = pool.tile([128, 8, 64], I32, tag="msa", name="msa")
            nc.vector.tensor_scalar(msa[:], m1[:], 512, None, op0=ADD_)
            nc.vector.tensor_scalar(msa[:], msa[:], 1023, None, op0=AND_)
            mcf = pool.tile([128, 8, 64], F32, tag="mcf", name="mcf")
            nc.vector.tensor_copy(mcf[:], mca[:])
            msf = pool.tile([128, 8, 64], F32, tag="msf", name="msf")
            nc.vector.tensor_copy(msf[:], msa[:])
            # g3: m2 = j * f(p)
            jgd = pool.tile([128, 1024], I32, tag="jgd", name="jgd")
            nc.vector.tensor_copy(jgd[:], jgi[:])
            m2 = pool.tile([128, 1024], I32, tag="m2", name="m2")
            nc.vector.tensor_tensor(m2[:], jgd[:],
                                    fgc[:].broadcast_to([128, 1024]), op=MUL_)
            ga_i = pool.tile([128, 1024], I32, tag="gai", name="gai")
            nc.vector.tensor_scalar(ga_i[:], m2[:], 256, None, op0=ADD_)
            nc.vector.tensor_scalar(ga_i[:], ga_i[:], 1023, None, op0=AND_)
            gs_i = pool.tile([128, 1024], I32, tag="gsi", name="gsi")
            nc.vector.tensor_scalar(gs_i[:], m2[:], 512, None, op0=ADD_)
            nc.vector.tensor_scalar(gs_i[:], gs_i[:], 1023, None, op0=AND_)
            gaf = pool.tile([128, 1024], F32, tag="gaf", name="gaf")
            nc.vector.tensor_copy(gaf[:], ga_i[:])
            gsf = pool.tile([128, 1024], F32, tag="gsf", name="gsf")
            nc.vector.tensor_copy(gsf[:], gs_i[:])

            # ACT: Sins (cc first — S1 waits on cc_t)
            cc_raw = pool.tile([128, 8, 128], BF16, tag="ccraw", name="ccraw")
            nc.scalar.activation(cc_raw[:, :, 0:64], mcf[:], SIN,
                                 bias=scl[:, 2:3], scale=S2PI)
            nc.scalar.activation(cc_raw[:, :, 64:128], msf[:], SIN,
                                 bias=scl[:, 2:3], scale=S2PI)
            g3a_raw = pool.tile([128, 1024], BF16, tag="g3ar", name="g3ar")
            nc.scalar.activation(g3a_raw[:], gaf[:], SIN,
                                 bias=scl[:, 2:3], scale=S2PI)
            g3b_raw = pool.tile([128, 1024], BF16, tag="g3br", name="g3br")
            act_g_last = nc.scalar.activation(g3b_raw[:], gsf[:], SIN,
                                              bias=scl[:, 2:3], scale=S2PI)

            # DVE: cc finalize (single producer for PE waits) + g3 scales
            cc_t = pool.tile([128, 8, 128], BF16, tag="cct", name="cct")
            nc.vector.tensor_copy(
                cc_t[:].rearrange("p c s -> p (c s)"),
                cc_raw[:].rearrange("p c s -> p (c s)"))
            nc.vector.tensor_copy(cc_t[:, :, 64:65], im0[:].unsqueeze(2))
            g3a_v = pool.tile([128, 1024], BF16, tag="g3a", name="g3a")
            nc.vector.tensor_scalar(g3a_v[:], g3a_raw[:], scl[:, 0:1], None,
                                    op0=MUL_)
            g3b_v = pool.tile([128, 1024], BF16, tag="g3b", name="g3b")
            nc.vector.tensor_scalar(g3b_v[:], g3b_raw[:], scl[:, 1:2], None,
                                    op0=MUL_)

            # ============ main pipeline ============
            # PE warmup: hold the clock until kt h0 lands (~8 junk mms)
            for w in range(8):
                nc.tensor.matmul(ps_junk[:], wz[:, :128], wz[:, 128:640],
                                 start=True, stop=True)

            # S1 h0 (ACT queue's chunks usually land first)
            h_order = [4, 5, 6, 7, 0, 1, 2, 3]
            for i, c in enumerate(h_order):
                nc.tensor.matmul(ps_kc0[:], cc_t[:, c, :], ktv[(c, 0)],
                                 start=(i == 0), stop=(i == 7))
            # T1 h0: PSUM copy on ACT (its Sins are done by now), PE
            # transposes, kc copy on DVE
            kcT0 = pool.tile([128, 512], BF16, tag="kcT0", name="kcT0")
            nc.scalar.copy(kcT0[:], ps_kc0[:])
            for c in range(4):
                nc.tensor.transpose(trall[:, c, :],
                                    kcT0[:, c * 128:(c + 1) * 128], id_v)
            kc_lo = pool.tile([128, 4, 128], BF16, tag="kclo", name="kclo")
            nc.vector.tensor_copy(
                kc_lo[:].rearrange("p c s -> p (c s)"),
                trall[:].rearrange("p c s -> p (c s)"))

            # S2 partial 0
            for c in range(4):
                nc.tensor.matmul(ps_db[:], kc_lo[:, c, :], dbt_v[:, c, :],
                                 start=(c == 0), stop=False)

            # clock-hold junk while kt h1 streams
            for w in range(3):
                nc.tensor.matmul(ps_junk[:, 0:256], wz[:, :128],
                                 wz[:, 128:384], start=True, stop=True)

            # S1 h1
            for i, c in enumerate(h_order):
                nc.tensor.matmul(ps_kc1[:], cc_t[:, c, :], ktv[(c, 1)],
                                 start=(i == 0), stop=(i == 7))
            # T1 h1
            kcT1 = pool.tile([128, 512], BF16, tag="kcT1", name="kcT1")
            nc.scalar.copy(kcT1[:], ps_kc1[:])
            for c in range(4):
                nc.tensor.transpose(trall2[:, c, :],
                                    kcT1[:, c * 128:(c + 1) * 128], id_v)
            kc_hi = pool.tile([128, 4, 128], BF16, tag="kchi", name="kchi")
            nc.vector.tensor_copy(
                kc_hi[:].rearrange("p c s -> p (c s)"),
                trall2[:].rearrange("p c s -> p (c s)"))

            # S2 partial 1
            for c in range(4):
                nc.tensor.matmul(ps_db[:], kc_hi[:, c, :], dbt_v[:, 4 + c, :],
                                 start=False, stop=(c == 3))

            # PW in [s, b] layout
            db_sb = pool.tile([128, 2 * B], BF16, tag="db", name="db")
            nc.vector.tensor_copy(db_sb[:], ps_db[:])
            dbsw = pool.tile([128, B], BF16, tag="dbsw", name="dbsw")
            nc.vector.tensor_copy(dbsw[0:64, :], db_sb[64:128, B:2 * B])
            nc.vector.tensor_copy(dbsw[64:128, :], db_sb[0:64, B:2 * B])
            ptA = pool.tile([128, B], BF16, tag="ptA", name="ptA")
            ptC2 = pool.tile([128, B], BF16, tag="ptC2", name="ptC2")
            nc.vector.tensor_mul(ptA[:], db_sb[:, 0:B], db_sb[:, B:2 * B])
            nc.vector.tensor_mul(ptC2[:], db_sb[:, 0:B], dbsw[:])

            # S4 per bank + cast + store
            out_lo = pool.tile([128, 512], BF16, tag="outlo", name="outlo")
            out_hi = pool.tile([128, 512], BF16, tag="outhi", name="outhi")
            stores = []
            nc.tensor.matmul(ps_out_lo[:], ptA[:], g3a_v[:, 0:512],
                             start=True, stop=False)
            nc.tensor.matmul(ps_out_lo[:], ptC2[:], g3b_v[:, 0:512],
                             start=False, stop=True)
            cp_lo = nc.scalar.copy(out_lo[:], ps_out_lo[:])
            stores.append(nc.sync.dma_start(out[:, :256],
                                            out_lo.bitcast(F32)[:, :]))
            nc.tensor.matmul(ps_out_hi[:], ptA[:], g3a_v[:, 512:1024],
                             start=True, stop=False)
            last_mm = nc.tensor.matmul(ps_out_hi[:], ptC2[:],
                                       g3b_v[:, 512:1024],
                                       start=False, stop=True)
            cp_hi = nc.vector.tensor_copy(out_hi[:], ps_out_hi[:])
            stores.append(nc.scalar.dma_start(out[:, 256:],
                                              out_hi.bitcast(F32)[:, :]))

            # tail: absorb every outstanding tick into SP's clock
            prev = None
            for dep in [*in_dmas, memset_h, gp_last, act_g_last, *stores,
                        last_mm, cp_lo, cp_hi]:
                dr = nc.sync.drain(fusable=False)
                add_dep_helper(dr.ins, dep.ins, sync=True,
                               reason="tail: absorb tick into SP clock")
                if prev is not None:
                    add_dep_helper(dr.ins, prev.ins, sync=False,
                                   reason="tail: keep drain chain ordered")
                prev = dr

    return nc


def _bf16_pack(a):
    """float32 (P, W) -> bf16 packed two-per-word as float32 (P, W//2)."""
    bf = np.ascontiguousarray(np.asarray(a, np.float32).astype(ml_dtypes.bfloat16))
    return bf.view(np.uint8).reshape(bf.shape[0], -1).view(np.float32)


def _partition_pack(a):
    """(n*128, W) -> (128, n*W): row p = concat of chunk rows p."""
    r, w = a.shape
    n = r // 128
    return np.ascontiguousarray(
        a.reshape(n, 128, w).transpose(1, 0, 2).reshape(128, n * w))


def _aux_for_core(core):
    p = np.arange(128)
    c = np.arange(8)
    j = c[None, :] * 128 + p[:, None]                 # [128, 8]
    i32 = np.zeros((128, AUXW), np.int32)
    i32[:, 0:8] = (64 * core * j) % 1024
    fg = 64 * core + (p % 64)
    sa = np.where(p < 64, 4.0 / N, -4.0 / N).astype(np.float32)
    # g3b raw value is sin(th+pi) = -sin(th); C = -w sin(th) -> scale +w
    sb = np.full(128, 4.0 / N, np.float32)
    if core == 0:
        im0 = np.cos(np.pi * j).astype(np.float32)
        fg = fg.copy(); fg[64] = 512
        sa = sa.copy(); sa[0] = 2.0 / N; sa[64] = 2.0 / N
        sb = sb.copy(); sb[0] = 0.0; sb[64] = 0.0
    else:
        im0 = -np.sin(2.0 * np.pi * j * (64 * core) / N).astype(np.float32)
    i32[:, 8] = fg
    auxf = i32.view(np.float32).copy()
    auxf[:, 9] = sa
    auxf[:, 10] = sb
    auxf[:, 11] = np.pi
    auxf[:, 12:20] = im0
    return np.ascontiguousarray(auxf)


def kernel(des, body, kernel):
    global LAST_RESULT
    K = np.asarray(kernel, dtype=np.float32)
    des = np.asarray(des, dtype=np.float32)
    body = np.asarray(body, dtype=np.float32)

    ktb = K.T.astype(ml_dtypes.bfloat16)                # (1024 j, 1024 k)
    def ktpk(c, h):
        blk = np.ascontiguousarray(
            ktb[c * 128:(c + 1) * 128, h * 512:(h + 1) * 512], np.float32)
        return _bf16_pack(blk)                          # (128, 256) words

    id_pk = _bf16_pack(np.eye(128, dtype=np.float32))   # (128, 64) words
    dbt_np = np.concatenate([des.T, body.T], axis=1)    # (1024, 256)
    dbt_pk = _partition_pack(_bf16_pack(dbt_np))        # (128, 1024) words

    sp1v = np.ascontiguousarray(np.concatenate(
        [id_pk] + [ktpk(c, 0) for c in range(4)], axis=1))
    sp2v = np.ascontiguousarray(np.concatenate(
        [ktpk(c, 1) for c in range(4)], axis=1))
    ac1v = np.ascontiguousarray(np.concatenate(
        [ktpk(4 + c, 0) for c in range(4)], axis=1))
    ac2v = np.ascontiguousarray(np.concatenate(
        [ktpk(4 + c, 1) for c in range(4)], axis=1))

    in_maps = []
    for core in range(N_CORES):
        in_maps.append({
            "sp1": sp1v, "sp2": sp2v, "ac1": ac1v, "ac2": ac2v,
            "aux": _aux_for_core(core), "dbt": dbt_pk,
        })

    if "nc" not in _nc_cache:
        _nc_cache["nc"] = _build_nc()
    nc = _nc_cache["nc"]

    res = run_bass_kernel_spmd(nc, in_maps, list(range(N_CORES)))
    LAST_RESULT = res
    out = np.zeros((B, N), dtype=np.float32)
    for r in res.results:
        w = np.ascontiguousarray(np.asarray(r["out"], np.float32))
        bf = w.view(np.uint8).reshape(B, -1).view(ml_dtypes.bfloat16)
        out += bf.astype(np.float32)
    return out


# revision 16
# speedup vs baseline: 1.1801x; 1.0992x over previous
r"""Circulant layer kernel for Trainium2 (8 NeuronCores) — v7.

Math (same as v2): reference computes mv1 + mv2 = 2 * circconv(d, b)
with d = des @ K, b = body @ K.  Real-input half-spectrum DFT: cores
0..7 own freqs f = 64c..64c+63; Nyquist f=512 rides core 0's slot-0
imaginary column with the generalized 3-product inverse (G3).

v7 vs v3: the input stream is ordered by when each tensor is needed.
dbt (needed at S2 partial 1, ~24us) rides the kt h1 HWDGE DMAs; g3a/g3b
(needed only at S4, ~26us) move to the slow SWDGE queue behind cc.
Keeping non-DMA engines QUIET during the stream matters: junk matmuls /
generation ALU measurably throttle DMA ingress (245 -> 140GB/s).

v3 structural changes vs v2 (40.9us -> 38.1 measured):
  * K^T streams on BOTH hardware DMA queues (SP + ACT), split by
    j-chunk pairs and k-halves; cc/dbt ride the gpsimd SWDGE queue.
    (v2 put all of kt on one queue at ~190GB/s — the single-queue
    stream, not PE, set the critical path.)
  * k-half phasing: S1 (KC^T = CC^T K^T) accumulates k-half 0 in PSUM
    bank 0 and k-half 1 in bank 1, so T1/S2 for half 0 run while
    half 1 is still streaming in.
  * The pointwise spectral products are computed directly in [s, b]
    layout from S2's output (DVE ops with partition-base-shifted
    operands — verified on HW), eliminating T2, T3 and their staging
    copies entirely:
      ptA[p, b]        = db[p, b] * db[p, B+b]          (p = 0..127)
      ptC2[p, b]       = db[p, b] * db[(p+64)%128, B+b] (two half ops)
    ptA/ptC2 feed S4 as stationaries with G3a / duplicated-C moving.
  * S4 + cast + store issue per 512-col PSUM bank as soon as ready.

Fixed costs measured by probe: ~8.3us preamble, ~2us DMA issue->land,
~2us store issue->tick, ~8.3us after last store tick.
"""

import numpy as np
import ml_dtypes

import concourse.bass as bass
import concourse.mybir as mybir
import concourse.tile as tile
from concourse.bass_utils import run_bass_kernel_spmd
from concourse.tile_rust import add_dep_helper

B = 128        # batch
D_IN = 1024    # input feature dim (contraction k)
N = 1024       # output feature dim (conv length j)
N_CORES = 8
FPC = 64       # complex frequency slots per core
S = 2 * FPC    # 128 freq columns per core: [0:64]=re(cos), [64:128]=im(-sin)

F32 = mybir.dt.float32
BF16 = mybir.dt.bfloat16

LAST_RESULT = None
_nc_cache = {}


def _build_nc():
    nc = bass.Bass(target_bir_lowering=True)

    # --- DRAM params (bf16 packed two-per-f32-word) ---
    # SP queue: [id | kt h0 c0c1] [kt h0 c2c3] [kt h1 c0c1] [kt h1 c2c3] [g3a]
    # ACT queue: [kt h0 c4c5] [kt h0 c6c7] [kt h1 c4c5] [kt h1 c6c7] [g3b2]
    # GP queue: [cc] [dbt]
    # each kt (c,h) block: [128, 512] bf16 = 256 f32 words; pairs = 512 words
    sp1 = nc.declare_dram_parameter("sp1", [128, 64 + 1024], F32, False)
    sp2 = nc.declare_dram_parameter("sp2", [128, 1536], F32, False)
    ac1 = nc.declare_dram_parameter("ac1", [128, 1024], F32, False)
    ac2 = nc.declare_dram_parameter("ac2", [128, 1536], F32, False)
    cc = nc.declare_dram_parameter("cc", [128, 512], F32, False)
    g3a = nc.declare_dram_parameter("g3a", [128, 512], F32, False)
    g3b = nc.declare_dram_parameter("g3b", [128, 512], F32, False)
    out = nc.declare_dram_parameter("out", [B, N // 2], F32, isOutput=True)

    with tile.TileContext(nc) as tc:
        with (
            tc.tile_pool(name="main", bufs=1) as pool,
            tc.tile_pool(name="psum", bufs=1, space="PSUM") as pp,
        ):
            # ---- input DMAs, phase-ordered per queue ----
            sp1_sb = pool.tile([128, 64 + 1024], F32, tag="sp1", name="sp1")
            sp2_sb = pool.tile([128, 1536], F32, tag="sp2", name="sp2")
            ac1_sb = pool.tile([128, 1024], F32, tag="ac1", name="ac1")
            ac2_sb = pool.tile([128, 1536], F32, tag="ac2", name="ac2")
            cc_sb = pool.tile([128, 512], F32, tag="cc", name="cc")
            g3a_sb = pool.tile([128, 512], F32, tag="g3a", name="g3a")
            g3b_sb = pool.tile([128, 512], F32, tag="g3b", name="g3b")

            in_dmas = []
            in_dmas.append(nc.sync.dma_start(sp1_sb[:], sp1[:, :]))
            in_dmas.append(nc.sync.dma_start(sp2_sb[:], sp2[:, :]))
            in_dmas.append(nc.scalar.dma_start(ac1_sb[:], ac1[:, :]))
            in_dmas.append(nc.scalar.dma_start(ac2_sb[:], ac2[:, :]))
            in_dmas.append(nc.gpsimd.dma_start(cc_sb[:], cc[:, :]))
            in_dmas.append(nc.gpsimd.dma_start(g3a_sb[:], g3a[:, :]))
            in_dmas.append(nc.gpsimd.dma_start(g3b_sb[:], g3b[:, :]))

            # bf16 views
            id_v = sp1_sb.bitcast(BF16)[:, 0:128]
            # kt[c][h] -> [128, 512] bf16 view
            sp1v = sp1_sb.bitcast(BF16)
            sp2v = sp2_sb.bitcast(BF16)
            ac1v = ac1_sb.bitcast(BF16)
            ac2v = ac2_sb.bitcast(BF16)
            ktv = {}
            for c in range(4):
                ktv[(c, 0)] = sp1v[:, 128 + c * 512:128 + (c + 1) * 512]
                ktv[(c, 1)] = sp2v[:, c * 512:(c + 1) * 512]
                ktv[(4 + c, 0)] = ac1v[:, c * 512:(c + 1) * 512]
                ktv[(4 + c, 1)] = ac2v[:, c * 512:(c + 1) * 512]
            g3a_v = g3a_sb.bitcast(BF16)          # [128, 1024]
            g3b_v = g3b_sb.bitcast(BF16)          # [128, 1024]
            cc_v = cc_sb.bitcast(BF16).rearrange(
                "p (c s) -> p c s", c=8)          # [128, 8, 128]
            # dbt rides the tails of sp2/ac2: chunks 0-3 and 4-7
            dbt_lo = sp2v[:, 2048:3072].rearrange(
                "p (c w) -> p c w", c=4)          # [128, 4, 256]
            dbt_hi = ac2v[:, 2048:3072].rearrange(
                "p (c w) -> p c w", c=4)

            # ---- PSUM layout ----
            ps_kc0 = pp.tile([128, 512], F32, tag="pskc0", name="pskc0")
            ps_kc1 = pp.tile([128, 512], F32, tag="pskc1", name="pskc1")
            ps_db = pp.tile([128, 2 * B], F32, tag="psdb", name="psdb")
            trall = pp.tile([128, 4, 128], BF16, tag="trall", name="trall")
            trall2 = pp.tile([128, 4, 128], BF16, tag="trall2", name="trall2")
            ps_out_lo = pp.tile([128, 512], F32, tag="psoutl", name="psoutl")
            ps_out_hi = pp.tile([128, 512], F32, tag="psouth", name="psouth")

            # ---- PE warmup: junk matmuls into ps_out (S4 overwrites) ----
            wz = pool.tile([128, 640], BF16, tag="wz", name="wz")
            memset_h = nc.gpsimd.memset(wz[:], 0.0)
            for w in range(4):
                nc.tensor.matmul(ps_out_lo[:], wz[:, :128], wz[:, 128:640],
                                 start=True, stop=True)

            # ---- S1 phase 0: ps_kc0[s, k0:512] = sum_j cc[j,s]^T kt[j, h0] ----
            # mm order follows expected landing: SP pair (0,1), ACT (4,5),
            # SP (2,3), ACT (6,7)
            h0_order = [0, 1, 4, 5, 2, 3, 6, 7]
            for i, c in enumerate(h0_order):
                nc.tensor.matmul(ps_kc0[:], cc_v[:, c, :], ktv[(c, 0)],
                                 start=(i == 0), stop=(i == 7))

            # ---- T1 phase 0: transpose KC^T[:, 0:512] -> kc chunks 0..3 ----
            kcT0 = pool.tile([128, 512], BF16, tag="kcT0", name="kcT0")
            nc.scalar.copy(kcT0[:], ps_kc0[:])
            for c in range(4):
                nc.tensor.transpose(trall[:, c, :],
                                    kcT0[:, c * 128:(c + 1) * 128], id_v)
            kc_lo = pool.tile([128, 4, 128], BF16, tag="kclo", name="kclo")
            nc.vector.tensor_copy(
                kc_lo[:].rearrange("p c s -> p (c s)"),
                trall[:].rearrange("p c s -> p (c s)"))

            # ---- S1 phase 1 (first half): overlap with S2p0 setup ----
            h1_order = [0, 1, 4, 5, 2, 3, 6, 7]
            for i, c in enumerate(h1_order[:2]):
                nc.tensor.matmul(ps_kc1[:], cc_v[:, c, :], ktv[(c, 1)],
                                 start=(i == 0), stop=False)

            # ---- S2 partial 0: ps_db += kc[k0 chunks] @ dbt ----
            for c in range(4):
                nc.tensor.matmul(ps_db[:], kc_lo[:, c, :], dbt_lo[:, c, :],
                                 start=(c == 0), stop=False)

            # ---- S1 phase 1 (rest) ----
            for i, c in enumerate(h1_order[2:]):
                nc.tensor.matmul(ps_kc1[:], cc_v[:, c, :], ktv[(c, 1)],
                                 start=False, stop=(i == 5))

            # ---- T1 phase 1 ----
            kcT1 = pool.tile([128, 512], BF16, tag="kcT1", name="kcT1")
            nc.scalar.copy(kcT1[:], ps_kc1[:])
            for c in range(4):
                nc.tensor.transpose(trall2[:, c, :],
                                    kcT1[:, c * 128:(c + 1) * 128], id_v)
            # separate PSUM tile (trall2): a second read of the same PSUM
            # tile needs two sync waits, which compute instrs can't encode
            kc_hi = pool.tile([128, 4, 128], BF16, tag="kchi", name="kchi")
            nc.vector.tensor_copy(
                kc_hi[:].rearrange("p c s -> p (c s)"),
                trall2[:].rearrange("p c s -> p (c s)"))

            # ---- S2 partial 1 ----
            for c in range(4):
                nc.tensor.matmul(ps_db[:], kc_hi[:, c, :],
                                 dbt_hi[:, c, :],
                                 start=False, stop=(c == 3))

            # ---- PW in [s, b] layout (no transposes) ----
            db_sb = pool.tile([128, 2 * B], BF16, tag="db", name="db")
            nc.vector.tensor_copy(db_sb[:], ps_db[:])
            # partition-swapped copy of the B half (tensor_tensor requires
            # same start partition on all APs; tensor_copy does not)
            dbsw = pool.tile([128, B], BF16, tag="dbsw", name="dbsw")
            nc.vector.tensor_copy(dbsw[0:64, :], db_sb[64:128, B:2 * B])
            nc.vector.tensor_copy(dbsw[64:128, :], db_sb[0:64, B:2 * B])
            ptA = pool.tile([128, B], BF16, tag="ptA", name="ptA")
            ptC2 = pool.tile([128, B], BF16, tag="ptC2", name="ptC2")
            nc.vector.tensor_mul(ptA[:], db_sb[:, 0:B], db_sb[:, B:2 * B])
            nc.vector.tensor_mul(ptC2[:], db_sb[:, 0:B], dbsw[:])

            # ---- S4 per bank: out[b, j] = ptA^T G3a + ptC2^T G3b2 ----
            out_lo = pool.tile([128, 512], BF16, tag="outlo", name="outlo")
            out_hi = pool.tile([128, 512], BF16, tag="outhi", name="outhi")
            stores = []
            nc.tensor.matmul(ps_out_lo[:], ptA[:], g3a_v[:, 0:512],
                             start=True, stop=False)
            nc.tensor.matmul(ps_out_lo[:], ptC2[:], g3b_v[:, 0:512],
                             start=False, stop=True)
            cp_lo = nc.scalar.copy(out_lo[:], ps_out_lo[:])
            stores.append(nc.sync.dma_start(out[:, :256],
                                            out_lo.bitcast(F32)[:, :]))
            nc.tensor.matmul(ps_out_hi[:], ptA[:], g3a_v[:, 512:1024],
                             start=True, stop=False)
            last_mm = nc.tensor.matmul(ps_out_hi[:], ptC2[:],
                                       g3b_v[:, 512:1024],
                                       start=False, stop=True)
            cp_hi = nc.vector.tensor_copy(out_hi[:], ps_out_hi[:])
            stores.append(nc.scalar.dma_start(out[:, 256:],
                                              out_hi.bitcast(F32)[:, :]))

            # ---- tail: absorb every outstanding tick into SP's clock ----
            prev = None
            for dep in [*in_dmas, memset_h, *stores, last_mm, cp_lo, cp_hi]:
                dr = nc.sync.drain(fusable=False)
                add_dep_helper(dr.ins, dep.ins, sync=True,
                               reason="tail: absorb tick into SP clock")
                if prev is not None:
                    add_dep_helper(dr.ins, prev.ins, sync=False,
                                   reason="tail: keep drain chain ordered")
                prev = dr

    return nc


def _bf16_pack(a):
    """float32 (P, W) -> bf16 packed two-per-word as float32 (P, W//2)."""
    bf = np.ascontiguousarray(np.asarray(a, np.float32).astype(ml_dtypes.bfloat16))
    return bf.view(np.uint8).reshape(bf.shape[0], -1).view(np.float32)


def _partition_pack(a):
    """(n*128, W) -> (128, n*W): row p = concat of chunk rows p."""
    r, w = a.shape
    n = r // 128
    return np.ascontiguousarray(
        a.reshape(n, 128, w).transpose(1, 0, 2).reshape(128, n * w))


def _constants():
    """Per-core CC [N, S], G3a [128, N], G3b2 [128, N] float32."""
    j = np.arange(N, dtype=np.float64)
    alt = np.cos(np.pi * j)                     # (-1)^j
    ccs, g3as, g3bs = [], [], []
    for c in range(N_CORES):
        f = np.arange(c * FPC, (c + 1) * FPC, dtype=np.float64)
        ang = 2.0 * np.pi * np.outer(j, f) / N             # (j, t)
        cc_re = np.cos(ang)
        cc_im = -np.sin(ang)
        angT = ang.T                                        # (t, j)
        w = 4.0 / N
        A = w * np.cos(angT)                                # m0 rows
        Bm = -w * np.cos(angT)                              # m1 rows
        C = -w * np.sin(angT)                               # mC rows
        if c == 0:
            cc_im[:, 0] = alt                               # f=512 cos column
            A[0, :] = 2.0 / N                               # m0 = D0*B0
            Bm[0, :] = (2.0 / N) * alt                      # m1 = D512*B512
            C[0, :] = 0.0
        cc_full = np.concatenate([cc_re, cc_im], axis=1)    # (N, 128)
        ccs.append(np.ascontiguousarray(cc_full, np.float32))
        g3as.append(np.ascontiguousarray(
            np.concatenate([A, Bm], axis=0), np.float32))       # (128, N)
        g3bs.append(np.ascontiguousarray(
            np.concatenate([C, C], axis=0), np.float32))        # (128, N)
    return ccs, g3as, g3bs


def kernel(des, body, kernel):
    global LAST_RESULT
    K = np.asarray(kernel, dtype=np.float32)
    des = np.asarray(des, dtype=np.float32)
    body = np.asarray(body, dtype=np.float32)

    # K^T as bf16 blocks: block (c, h) = K^T[c*128:(c+1)*128, h*512:(h+1)*512]
    ktb = K.T.astype(ml_dtypes.bfloat16)                # (1024 j, 1024 k)
    def ktpk(c, h):
        blk = np.ascontiguousarray(
            ktb[c * 128:(c + 1) * 128, h * 512:(h + 1) * 512], np.float32)
        return _bf16_pack(blk)                          # (128, 256) words

    id_pk = _bf16_pack(np.eye(128, dtype=np.float32))   # (128, 64) words
    dbt_np = np.concatenate([des.T, body.T], axis=1)    # (1024, 256)
    dbt_pk = _partition_pack(_bf16_pack(dbt_np))        # (128, 1024) words

    ccs, g3as, g3bs = _constants()
    in_maps = []
    for c in range(N_CORES):
        cc_pk = _partition_pack(_bf16_pack(ccs[c]))     # (128, 512) words
        m = {
            "sp1": np.ascontiguousarray(np.concatenate(
                [id_pk, ktpk(0, 0), ktpk(1, 0), ktpk(2, 0), ktpk(3, 0)],
                axis=1)),
            "sp2": np.ascontiguousarray(np.concatenate(
                [ktpk(0, 1), ktpk(1, 1), ktpk(2, 1), ktpk(3, 1),
                 dbt_pk[:, 0:512]], axis=1)),
            "ac1": np.ascontiguousarray(np.concatenate(
                [ktpk(4, 0), ktpk(5, 0), ktpk(6, 0), ktpk(7, 0)], axis=1)),
            "ac2": np.ascontiguousarray(np.concatenate(
                [ktpk(4, 1), ktpk(5, 1), ktpk(6, 1), ktpk(7, 1),
                 dbt_pk[:, 512:1024]], axis=1)),
            "cc": cc_pk,
            "g3a": np.ascontiguousarray(_bf16_pack(g3as[c])),
            "g3b": np.ascontiguousarray(_bf16_pack(g3bs[c])),
        }
        in_maps.append(m)

    if "nc" not in _nc_cache:
        _nc_cache["nc"] = _build_nc()
    nc = _nc_cache["nc"]

    res = run_bass_kernel_spmd(nc, in_maps, list(range(N_CORES)))
    LAST_RESULT = res
    out = np.zeros((B, N), dtype=np.float32)
    for r in res.results:
        w = np.ascontiguousarray(np.asarray(r["out"], np.float32))
        bf = w.view(np.uint8).reshape(B, -1).view(ml_dtypes.bfloat16)
        out += bf.astype(np.float32)
    return out


# revision 17
# speedup vs baseline: 1.2052x; 1.0213x over previous
r"""Circulant layer kernel for Trainium2 (8 NeuronCores) — v7.

Math (same as v2): reference computes mv1 + mv2 = 2 * circconv(d, b)
with d = des @ K, b = body @ K.  Real-input half-spectrum DFT: cores
0..7 own freqs f = 64c..64c+63; Nyquist f=512 rides core 0's slot-0
imaginary column with the generalized 3-product inverse (G3).

v7 vs v3: the input stream is ordered by when each tensor is needed.
dbt (needed at S2 partial 1, ~24us) rides the kt h1 HWDGE DMAs; g3a/g3b
(needed only at S4, ~26us) move to the slow SWDGE queue behind cc.
Keeping non-DMA engines QUIET during the stream matters: junk matmuls /
generation ALU measurably throttle DMA ingress (245 -> 140GB/s).

v3 structural changes vs v2 (40.9us -> 38.1 measured):
  * K^T streams on BOTH hardware DMA queues (SP + ACT), split by
    j-chunk pairs and k-halves; cc/dbt ride the gpsimd SWDGE queue.
    (v2 put all of kt on one queue at ~190GB/s — the single-queue
    stream, not PE, set the critical path.)
  * k-half phasing: S1 (KC^T = CC^T K^T) accumulates k-half 0 in PSUM
    bank 0 and k-half 1 in bank 1, so T1/S2 for half 0 run while
    half 1 is still streaming in.
  * The pointwise spectral products are computed directly in [s, b]
    layout from S2's output (DVE ops with partition-base-shifted
    operands — verified on HW), eliminating T2, T3 and their staging
    copies entirely:
      ptA[p, b]        = db[p, b] * db[p, B+b]          (p = 0..127)
      ptC2[p, b]       = db[p, b] * db[(p+64)%128, B+b] (two half ops)
    ptA/ptC2 feed S4 as stationaries with G3a / duplicated-C moving.
  * S4 + cast + store issue per 512-col PSUM bank as soon as ready.

Fixed costs measured by probe: ~8.3us preamble, ~2us DMA issue->land,
~2us store issue->tick, ~8.3us after last store tick.
"""

import numpy as np
import ml_dtypes

import concourse.bass as bass
import concourse.mybir as mybir
import concourse.tile as tile
from concourse.bass_utils import run_bass_kernel_spmd
from concourse.tile_rust import add_dep_helper

B = 128        # batch
D_IN = 1024    # input feature dim (contraction k)
N = 1024       # output feature dim (conv length j)
N_CORES = 8
FPC = 64       # complex frequency slots per core
S = 2 * FPC    # 128 freq columns per core: [0:64]=re(cos), [64:128]=im(-sin)

F32 = mybir.dt.float32
BF16 = mybir.dt.bfloat16

LAST_RESULT = None
_nc_cache = {}


def _build_nc():
    nc = bass.Bass(target_bir_lowering=True)

    # --- DRAM params (bf16 packed two-per-f32-word) ---
    # SP queue: [id | kt h0 c0c1] [kt h0 c2c3] [kt h1 c0c1] [kt h1 c2c3] [g3a]
    # ACT queue: [kt h0 c4c5] [kt h0 c6c7] [kt h1 c4c5] [kt h1 c6c7] [g3b2]
    # GP queue: [cc] [dbt]
    # each kt (c,h) block: [128, 512] bf16 = 256 f32 words; pairs = 512 words
    sp1 = nc.declare_dram_parameter("sp1", [128, 64 + 1024], F32, False)
    sp2 = nc.declare_dram_parameter("sp2", [128, 1536], F32, False)
    ac1 = nc.declare_dram_parameter("ac1", [128, 1024], F32, False)
    ac2 = nc.declare_dram_parameter("ac2", [128, 1536], F32, False)
    cc = nc.declare_dram_parameter("cc", [128, 512], F32, False)
    g3a = nc.declare_dram_parameter("g3a", [128, 512], F32, False)
    g3b = nc.declare_dram_parameter("g3b", [64, 512], F32, False)
    out = nc.declare_dram_parameter("out", [B, N // 2], F32, isOutput=True)

    with tile.TileContext(nc) as tc:
        with (
            tc.tile_pool(name="main", bufs=1) as pool,
            tc.tile_pool(name="psum", bufs=1, space="PSUM") as pp,
        ):
            # ---- input DMAs, phase-ordered per queue ----
            sp1_sb = pool.tile([128, 64 + 1024], F32, tag="sp1", name="sp1")
            sp2_sb = pool.tile([128, 1536], F32, tag="sp2", name="sp2")
            ac1_sb = pool.tile([128, 1024], F32, tag="ac1", name="ac1")
            ac2_sb = pool.tile([128, 1536], F32, tag="ac2", name="ac2")
            cc_sb = pool.tile([128, 512], F32, tag="cc", name="cc")
            g3a_sb = pool.tile([128, 512], F32, tag="g3a", name="g3a")
            g3b_sb = pool.tile([128, 512], F32, tag="g3b", name="g3b")

            in_dmas = []
            in_dmas.append(nc.sync.dma_start(sp1_sb[:], sp1[:, :]))
            in_dmas.append(nc.sync.dma_start(sp2_sb[:], sp2[:, :]))
            in_dmas.append(nc.sync.dma_start(g3a_sb[:], g3a[:, :]))
            in_dmas.append(nc.scalar.dma_start(ac1_sb[:], ac1[:, :]))
            in_dmas.append(nc.scalar.dma_start(ac2_sb[:], ac2[:, :]))
            in_dmas.append(nc.gpsimd.dma_start(cc_sb[:], cc[:, :]))
            in_dmas.append(nc.gpsimd.dma_start(g3b_sb[0:64, :], g3b[:, :]))

            # bf16 views
            id_v = sp1_sb.bitcast(BF16)[:, 0:128]
            # kt[c][h] -> [128, 512] bf16 view
            sp1v = sp1_sb.bitcast(BF16)
            sp2v = sp2_sb.bitcast(BF16)
            ac1v = ac1_sb.bitcast(BF16)
            ac2v = ac2_sb.bitcast(BF16)
            ktv = {}
            for c in range(4):
                ktv[(c, 0)] = sp1v[:, 128 + c * 512:128 + (c + 1) * 512]
                ktv[(c, 1)] = sp2v[:, c * 512:(c + 1) * 512]
                ktv[(4 + c, 0)] = ac1v[:, c * 512:(c + 1) * 512]
                ktv[(4 + c, 1)] = ac2v[:, c * 512:(c + 1) * 512]
            g3a_v = g3a_sb.bitcast(BF16)          # [128, 1024]
            # g3b = [C; C]: only rows 0-63 are DMA'd; duplicate on DVE
            g3b_v = g3b_sb.bitcast(BF16)          # [128, 1024]
            cc_v = cc_sb.bitcast(BF16).rearrange(
                "p (c s) -> p c s", c=8)          # [128, 8, 128]
            # dbt rides the tails of sp2/ac2: chunks 0-3 and 4-7
            dbt_lo = sp2v[:, 2048:3072].rearrange(
                "p (c w) -> p c w", c=4)          # [128, 4, 256]
            dbt_hi = ac2v[:, 2048:3072].rearrange(
                "p (c w) -> p c w", c=4)

            # ---- PSUM layout ----
            ps_kc0 = pp.tile([128, 512], F32, tag="pskc0", name="pskc0")
            ps_kc1 = pp.tile([128, 512], F32, tag="pskc1", name="pskc1")
            ps_db = pp.tile([128, 2 * B], F32, tag="psdb", name="psdb")
            trall = pp.tile([128, 4, 128], BF16, tag="trall", name="trall")
            trall2 = pp.tile([128, 4, 128], BF16, tag="trall2", name="trall2")
            ps_out_lo = pp.tile([128, 512], F32, tag="psoutl", name="psoutl")
            ps_out_hi = pp.tile([128, 512], F32, tag="psouth", name="psouth")

            # ---- PE warmup: junk matmuls into ps_out (S4 overwrites) ----
            wz = pool.tile([128, 640], BF16, tag="wz", name="wz")
            memset_h = nc.gpsimd.memset(wz[:], 0.0)
            for w in range(4):
                nc.tensor.matmul(ps_out_lo[:], wz[:, :128], wz[:, 128:640],
                                 start=True, stop=True)

            # ---- S1 phase 0: ps_kc0[s, k0:512] = sum_j cc[j,s]^T kt[j, h0] ----
            # mm order follows expected landing: SP pair (0,1), ACT (4,5),
            # SP (2,3), ACT (6,7)
            h0_order = [0, 1, 4, 5, 2, 3, 6, 7]
            for i, c in enumerate(h0_order):
                nc.tensor.matmul(ps_kc0[:], cc_v[:, c, :], ktv[(c, 0)],
                                 start=(i == 0), stop=(i == 7))

            # ---- T1 phase 0: transpose KC^T[:, 0:512] -> kc chunks 0..3 ----
            kcT0 = pool.tile([128, 512], BF16, tag="kcT0", name="kcT0")
            nc.scalar.copy(kcT0[:], ps_kc0[:])
            for c in range(4):
                nc.tensor.transpose(trall[:, c, :],
                                    kcT0[:, c * 128:(c + 1) * 128], id_v)
            kc_lo = pool.tile([128, 4, 128], BF16, tag="kclo", name="kclo")
            nc.vector.tensor_copy(
                kc_lo[:].rearrange("p c s -> p (c s)"),
                trall[:].rearrange("p c s -> p (c s)"))

            # ---- S1 phase 1 (first half): overlap with S2p0 setup ----
            h1_order = [0, 1, 4, 5, 2, 3, 6, 7]
            for i, c in enumerate(h1_order[:2]):
                nc.tensor.matmul(ps_kc1[:], cc_v[:, c, :], ktv[(c, 1)],
                                 start=(i == 0), stop=False)

            # ---- S2 partial 0: ps_db += kc[k0 chunks] @ dbt ----
            for c in range(4):
                nc.tensor.matmul(ps_db[:], kc_lo[:, c, :], dbt_lo[:, c, :],
                                 start=(c == 0), stop=False)

            # ---- S1 phase 1 (rest) ----
            for i, c in enumerate(h1_order[2:]):
                nc.tensor.matmul(ps_kc1[:], cc_v[:, c, :], ktv[(c, 1)],
                                 start=False, stop=(i == 5))

            # ---- T1 phase 1 ----
            kcT1 = pool.tile([128, 512], BF16, tag="kcT1", name="kcT1")
            nc.scalar.copy(kcT1[:], ps_kc1[:])
            for c in range(4):
                nc.tensor.transpose(trall2[:, c, :],
                                    kcT1[:, c * 128:(c + 1) * 128], id_v)
            # separate PSUM tile (trall2): a second read of the same PSUM
            # tile needs two sync waits, which compute instrs can't encode
            kc_hi = pool.tile([128, 4, 128], BF16, tag="kchi", name="kchi")
            nc.vector.tensor_copy(
                kc_hi[:].rearrange("p c s -> p (c s)"),
                trall2[:].rearrange("p c s -> p (c s)"))

            # ---- S2 partial 1 ----
            for c in range(4):
                nc.tensor.matmul(ps_db[:], kc_hi[:, c, :],
                                 dbt_hi[:, c, :],
                                 start=False, stop=(c == 3))

            # duplicate C into partitions 64-127 (partition-shifted copy)
            nc.vector.tensor_copy(g3b_v[64:128, :], g3b_v[0:64, :])

            # ---- PW in [s, b] layout (no transposes) ----
            db_sb = pool.tile([128, 2 * B], BF16, tag="db", name="db")
            nc.vector.tensor_copy(db_sb[:], ps_db[:])
            # partition-swapped copy of the B half (tensor_tensor requires
            # same start partition on all APs; tensor_copy does not)
            dbsw = pool.tile([128, B], BF16, tag="dbsw", name="dbsw")
            nc.vector.tensor_copy(dbsw[0:64, :], db_sb[64:128, B:2 * B])
            nc.vector.tensor_copy(dbsw[64:128, :], db_sb[0:64, B:2 * B])
            ptA = pool.tile([128, B], BF16, tag="ptA", name="ptA")
            ptC2 = pool.tile([128, B], BF16, tag="ptC2", name="ptC2")
            nc.vector.tensor_mul(ptA[:], db_sb[:, 0:B], db_sb[:, B:2 * B])
            nc.vector.tensor_mul(ptC2[:], db_sb[:, 0:B], dbsw[:])

            # ---- S4 per bank: out[b, j] = ptA^T G3a + ptC2^T G3b2 ----
            out_lo = pool.tile([128, 512], BF16, tag="outlo", name="outlo")
            out_hi = pool.tile([128, 512], BF16, tag="outhi", name="outhi")
            stores = []
            nc.tensor.matmul(ps_out_lo[:], ptA[:], g3a_v[:, 0:512],
                             start=True, stop=False)
            nc.tensor.matmul(ps_out_lo[:], ptC2[:], g3b_v[:, 0:512],
                             start=False, stop=True)
            cp_lo = nc.scalar.copy(out_lo[:], ps_out_lo[:])
            stores.append(nc.sync.dma_start(out[:, :256],
                                            out_lo.bitcast(F32)[:, :]))
            nc.tensor.matmul(ps_out_hi[:], ptA[:], g3a_v[:, 512:1024],
                             start=True, stop=False)
            last_mm = nc.tensor.matmul(ps_out_hi[:], ptC2[:],
                                       g3b_v[:, 512:1024],
                                       start=False, stop=True)
            cp_hi = nc.vector.tensor_copy(out_hi[:], ps_out_hi[:])
            stores.append(nc.scalar.dma_start(out[:, 256:],
                                              out_hi.bitcast(F32)[:, :]))

            # ---- tail: absorb every outstanding tick into SP's clock ----
            prev = None
            for dep in [*in_dmas, memset_h, *stores, last_mm, cp_lo, cp_hi]:
                dr = nc.sync.drain(fusable=False)
                add_dep_helper(dr.ins, dep.ins, sync=True,
                               reason="tail: absorb tick into SP clock")
                if prev is not None:
                    add_dep_helper(dr.ins, prev.ins, sync=False,
                                   reason="tail: keep drain chain ordered")
                prev = dr

    return nc


def _bf16_pack(a):
    """float32 (P, W) -> bf16 packed two-per-word as float32 (P, W//2)."""
    bf = np.ascontiguousarray(np.asarray(a, np.float32).astype(ml_dtypes.bfloat16))
    return bf.view(np.uint8).reshape(bf.shape[0], -1).view(np.float32)


def _partition_pack(a):
    """(n*128, W) -> (128, n*W): row p = concat of chunk rows p."""
    r, w = a.shape
    n = r // 128
    return np.ascontiguousarray(
        a.reshape(n, 128, w).transpose(1, 0, 2).reshape(128, n * w))


def _constants():
    """Per-core CC [N, S], G3a [128, N], G3b2 [128, N] float32."""
    j = np.arange(N, dtype=np.float64)
    alt = np.cos(np.pi * j)                     # (-1)^j
    ccs, g3as, g3bs = [], [], []
    for c in range(N_CORES):
        f = np.arange(c * FPC, (c + 1) * FPC, dtype=np.float64)
        ang = 2.0 * np.pi * np.outer(j, f) / N             # (j, t)
        cc_re = np.cos(ang)
        cc_im = -np.sin(ang)
        angT = ang.T                                        # (t, j)
        w = 4.0 / N
        A = w * np.cos(angT)                                # m0 rows
        Bm = -w * np.cos(angT)                              # m1 rows
        C = -w * np.sin(angT)                               # mC rows
        if c == 0:
            cc_im[:, 0] = alt                               # f=512 cos column
            A[0, :] = 2.0 / N                               # m0 = D0*B0
            Bm[0, :] = (2.0 / N) * alt                      # m1 = D512*B512
            C[0, :] = 0.0
        cc_full = np.concatenate([cc_re, cc_im], axis=1)    # (N, 128)
        ccs.append(np.ascontiguousarray(cc_full, np.float32))
        g3as.append(np.ascontiguousarray(
            np.concatenate([A, Bm], axis=0), np.float32))       # (128, N)
        g3bs.append(np.ascontiguousarray(
            np.concatenate([C, C], axis=0), np.float32))        # (128, N)
    return ccs, g3as, g3bs


def kernel(des, body, kernel):
    global LAST_RESULT
    K = np.asarray(kernel, dtype=np.float32)
    des = np.asarray(des, dtype=np.float32)
    body = np.asarray(body, dtype=np.float32)

    # K^T as bf16 blocks: block (c, h) = K^T[c*128:(c+1)*128, h*512:(h+1)*512]
    ktb = K.T.astype(ml_dtypes.bfloat16)                # (1024 j, 1024 k)
    def ktpk(c, h):
        blk = np.ascontiguousarray(
            ktb[c * 128:(c + 1) * 128, h * 512:(h + 1) * 512], np.float32)
        return _bf16_pack(blk)                          # (128, 256) words

    id_pk = _bf16_pack(np.eye(128, dtype=np.float32))   # (128, 64) words
    dbt_np = np.concatenate([des.T, body.T], axis=1)    # (1024, 256)
    dbt_pk = _partition_pack(_bf16_pack(dbt_np))        # (128, 1024) words

    ccs, g3as, g3bs = _constants()
    in_maps = []
    for c in range(N_CORES):
        cc_pk = _partition_pack(_bf16_pack(ccs[c]))     # (128, 512) words
        m = {
            "sp1": np.ascontiguousarray(np.concatenate(
                [id_pk, ktpk(0, 0), ktpk(1, 0), ktpk(2, 0), ktpk(3, 0)],
                axis=1)),
            "sp2": np.ascontiguousarray(np.concatenate(
                [ktpk(0, 1), ktpk(1, 1), ktpk(2, 1), ktpk(3, 1),
                 dbt_pk[:, 0:512]], axis=1)),
            "ac1": np.ascontiguousarray(np.concatenate(
                [ktpk(4, 0), ktpk(5, 0), ktpk(6, 0), ktpk(7, 0)], axis=1)),
            "ac2": np.ascontiguousarray(np.concatenate(
                [ktpk(4, 1), ktpk(5, 1), ktpk(6, 1), ktpk(7, 1),
                 dbt_pk[:, 512:1024]], axis=1)),
            "cc": cc_pk,
            "g3a": np.ascontiguousarray(_bf16_pack(g3as[c])),
            "g3b": np.ascontiguousarray(_bf16_pack(g3bs[c][0:64])),
        }
        in_maps.append(m)

    if "nc" not in _nc_cache:
        _nc_cache["nc"] = _build_nc()
    nc = _nc_cache["nc"]

    res = run_bass_kernel_spmd(nc, in_maps, list(range(N_CORES)))
    LAST_RESULT = res
    out = np.zeros((B, N), dtype=np.float32)
    for r in res.results:
        w = np.ascontiguousarray(np.asarray(r["out"], np.float32))
        bf = w.view(np.uint8).reshape(B, -1).view(ml_dtypes.bfloat16)
        out += bf.astype(np.float32)
    return out


# revision 18
# speedup vs baseline: 1.2256x; 1.0169x over previous
r"""Circulant layer kernel for Trainium2 (8 NeuronCores) — v7.

Math (same as v2): reference computes mv1 + mv2 = 2 * circconv(d, b)
with d = des @ K, b = body @ K.  Real-input half-spectrum DFT: cores
0..7 own freqs f = 64c..64c+63; Nyquist f=512 rides core 0's slot-0
imaginary column with the generalized 3-product inverse (G3).

v7 vs v3: the input stream is ordered by when each tensor is needed.
dbt (needed at S2 partial 1, ~24us) rides the kt h1 HWDGE DMAs; g3a/g3b
(needed only at S4, ~26us) move to the slow SWDGE queue behind cc.
Keeping non-DMA engines QUIET during the stream matters: junk matmuls /
generation ALU measurably throttle DMA ingress (245 -> 140GB/s).

v3 structural changes vs v2 (40.9us -> 38.1 measured):
  * K^T streams on BOTH hardware DMA queues (SP + ACT), split by
    j-chunk pairs and k-halves; cc/dbt ride the gpsimd SWDGE queue.
    (v2 put all of kt on one queue at ~190GB/s — the single-queue
    stream, not PE, set the critical path.)
  * k-half phasing: S1 (KC^T = CC^T K^T) accumulates k-half 0 in PSUM
    bank 0 and k-half 1 in bank 1, so T1/S2 for half 0 run while
    half 1 is still streaming in.
  * The pointwise spectral products are computed directly in [s, b]
    layout from S2's output (DVE ops with partition-base-shifted
    operands — verified on HW), eliminating T2, T3 and their staging
    copies entirely:
      ptA[p, b]        = db[p, b] * db[p, B+b]          (p = 0..127)
      ptC2[p, b]       = db[p, b] * db[(p+64)%128, B+b] (two half ops)
    ptA/ptC2 feed S4 as stationaries with G3a / duplicated-C moving.
  * S4 + cast + store issue per 512-col PSUM bank as soon as ready.

Fixed costs measured by probe: ~8.3us preamble, ~2us DMA issue->land,
~2us store issue->tick, ~8.3us after last store tick.
"""

import numpy as np
import ml_dtypes

import concourse.bass as bass
import concourse.mybir as mybir
import concourse.tile as tile
from concourse.bass_utils import run_bass_kernel_spmd
from concourse.tile_rust import add_dep_helper

B = 128        # batch
D_IN = 1024    # input feature dim (contraction k)
N = 1024       # output feature dim (conv length j)
N_CORES = 8
FPC = 64       # complex frequency slots per core
S = 2 * FPC    # 128 freq columns per core: [0:64]=re(cos), [64:128]=im(-sin)

F32 = mybir.dt.float32
BF16 = mybir.dt.bfloat16

LAST_RESULT = None
_nc_cache = {}


def _build_nc():
    nc = bass.Bass(target_bir_lowering=True)

    # --- DRAM params (bf16 packed two-per-f32-word) ---
    # SP queue: [id | kt h0 c0c1] [kt h0 c2c3] [kt h1 c0c1] [kt h1 c2c3] [g3a]
    # ACT queue: [kt h0 c4c5] [kt h0 c6c7] [kt h1 c4c5] [kt h1 c6c7] [g3b2]
    # GP queue: [cc] [dbt]
    # each kt (c,h) block: [128, 512] bf16 = 256 f32 words; pairs = 512 words
    sp1 = nc.declare_dram_parameter("sp1", [128, 64 + 1024], F32, False)
    sp2 = nc.declare_dram_parameter("sp2", [128, 1024], F32, False)
    ac1 = nc.declare_dram_parameter("ac1", [128, 1024], F32, False)
    ac2 = nc.declare_dram_parameter("ac2", [128, 2048], F32, False)
    cc = nc.declare_dram_parameter("cc", [128, 512], F32, False)
    g3a = nc.declare_dram_parameter("g3a", [128, 512], F32, False)
    g3b = nc.declare_dram_parameter("g3b", [64, 512], F32, False)
    out = nc.declare_dram_parameter("out", [B, N // 2], F32, isOutput=True)

    with tile.TileContext(nc) as tc:
        with (
            tc.tile_pool(name="main", bufs=1) as pool,
            tc.tile_pool(name="psum", bufs=1, space="PSUM") as pp,
        ):
            # ---- input DMAs, phase-ordered per queue ----
            sp1_sb = pool.tile([128, 64 + 1024], F32, tag="sp1", name="sp1")
            sp2_sb = pool.tile([128, 1024], F32, tag="sp2", name="sp2")
            ac1_sb = pool.tile([128, 1024], F32, tag="ac1", name="ac1")
            ac2_sb = pool.tile([128, 2048], F32, tag="ac2", name="ac2")
            cc_sb = pool.tile([128, 512], F32, tag="cc", name="cc")
            g3a_sb = pool.tile([128, 512], F32, tag="g3a", name="g3a")
            g3b_sb = pool.tile([128, 512], F32, tag="g3b", name="g3b")

            in_dmas = []
            in_dmas.append(nc.sync.dma_start(sp1_sb[:], sp1[:, :]))
            in_dmas.append(nc.sync.dma_start(sp2_sb[:], sp2[:, :]))
            in_dmas.append(nc.sync.dma_start(g3a_sb[:], g3a[:, :]))
            in_dmas.append(nc.scalar.dma_start(ac1_sb[:], ac1[:, :]))
            in_dmas.append(nc.scalar.dma_start(ac2_sb[:], ac2[:, :]))
            in_dmas.append(nc.gpsimd.dma_start(cc_sb[:], cc[:, :]))
            in_dmas.append(nc.gpsimd.dma_start(g3b_sb[0:64, :], g3b[:, :]))

            # bf16 views
            id_v = sp1_sb.bitcast(BF16)[:, 0:128]
            # kt[c][h] -> [128, 512] bf16 view
            sp1v = sp1_sb.bitcast(BF16)
            sp2v = sp2_sb.bitcast(BF16)
            ac1v = ac1_sb.bitcast(BF16)
            ac2v = ac2_sb.bitcast(BF16)
            ktv = {}
            for c in range(4):
                ktv[(c, 0)] = sp1v[:, 128 + c * 512:128 + (c + 1) * 512]
                ktv[(c, 1)] = sp2v[:, c * 512:(c + 1) * 512]
                ktv[(4 + c, 0)] = ac1v[:, c * 512:(c + 1) * 512]
                ktv[(4 + c, 1)] = ac2v[:, c * 512:(c + 1) * 512]
            g3a_v = g3a_sb.bitcast(BF16)          # [128, 1024]
            # g3b = [C; C]: only rows 0-63 are DMA'd; duplicate on DVE
            g3b_v = g3b_sb.bitcast(BF16)          # [128, 1024]
            cc_v = cc_sb.bitcast(BF16).rearrange(
                "p (c s) -> p c s", c=8)          # [128, 8, 128]
            # dbt rides the tail of ac2 (all 8 k-chunks)
            dbt_lo = ac2v[:, 2048:3072].rearrange(
                "p (c w) -> p c w", c=4)          # [128, 4, 256]
            dbt_hi = ac2v[:, 3072:4096].rearrange(
                "p (c w) -> p c w", c=4)

            # ---- PSUM layout ----
            ps_kc0 = pp.tile([128, 512], F32, tag="pskc0", name="pskc0")
            ps_kc1 = pp.tile([128, 512], F32, tag="pskc1", name="pskc1")
            ps_db = pp.tile([128, 2 * B], F32, tag="psdb", name="psdb")
            trall = pp.tile([128, 4, 128], BF16, tag="trall", name="trall")
            trall2 = pp.tile([128, 4, 128], BF16, tag="trall2", name="trall2")
            ps_out_lo = pp.tile([128, 512], F32, tag="psoutl", name="psoutl")
            ps_out_hi = pp.tile([128, 512], F32, tag="psouth", name="psouth")

            # ---- PE warmup: junk matmuls into ps_out (S4 overwrites) ----
            wz = pool.tile([128, 640], BF16, tag="wz", name="wz")
            memset_h = nc.gpsimd.memset(wz[:], 0.0)
            for w in range(4):
                nc.tensor.matmul(ps_out_lo[:], wz[:, :128], wz[:, 128:640],
                                 start=True, stop=True)

            # ---- S1 phase 0: ps_kc0[s, k0:512] = sum_j cc[j,s]^T kt[j, h0] ----
            # mm order follows expected landing: SP pair (0,1), ACT (4,5),
            # SP (2,3), ACT (6,7)
            h0_order = [0, 1, 4, 5, 2, 3, 6, 7]
            for i, c in enumerate(h0_order):
                nc.tensor.matmul(ps_kc0[:], cc_v[:, c, :], ktv[(c, 0)],
                                 start=(i == 0), stop=(i == 7))

            # ---- T1 phase 0: transpose KC^T[:, 0:512] -> kc chunks 0..3 ----
            kcT0 = pool.tile([128, 512], BF16, tag="kcT0", name="kcT0")
            nc.scalar.copy(kcT0[:], ps_kc0[:])
            for c in range(4):
                nc.tensor.transpose(trall[:, c, :],
                                    kcT0[:, c * 128:(c + 1) * 128], id_v)
            kc_lo = pool.tile([128, 4, 128], BF16, tag="kclo", name="kclo")
            nc.vector.tensor_copy(
                kc_lo[:].rearrange("p c s -> p (c s)"),
                trall[:].rearrange("p c s -> p (c s)"))

            # ---- S1 phase 1 (first half): overlap with S2p0 setup ----
            h1_order = [0, 1, 4, 5, 2, 3, 6, 7]
            for i, c in enumerate(h1_order[:2]):
                nc.tensor.matmul(ps_kc1[:], cc_v[:, c, :], ktv[(c, 1)],
                                 start=(i == 0), stop=False)

            # ---- S2 partial 0: ps_db += kc[k0 chunks] @ dbt ----
            for c in range(4):
                nc.tensor.matmul(ps_db[:], kc_lo[:, c, :], dbt_lo[:, c, :],
                                 start=(c == 0), stop=False)

            # ---- S1 phase 1 (rest) ----
            for i, c in enumerate(h1_order[2:]):
                nc.tensor.matmul(ps_kc1[:], cc_v[:, c, :], ktv[(c, 1)],
                                 start=False, stop=(i == 5))

            # ---- T1 phase 1 ----
            kcT1 = pool.tile([128, 512], BF16, tag="kcT1", name="kcT1")
            nc.scalar.copy(kcT1[:], ps_kc1[:])
            for c in range(4):
                nc.tensor.transpose(trall2[:, c, :],
                                    kcT1[:, c * 128:(c + 1) * 128], id_v)
            # separate PSUM tile (trall2): a second read of the same PSUM
            # tile needs two sync waits, which compute instrs can't encode
            kc_hi = pool.tile([128, 4, 128], BF16, tag="kchi", name="kchi")
            nc.vector.tensor_copy(
                kc_hi[:].rearrange("p c s -> p (c s)"),
                trall2[:].rearrange("p c s -> p (c s)"))

            # ---- S2 partial 1 ----
            for c in range(4):
                nc.tensor.matmul(ps_db[:], kc_hi[:, c, :],
                                 dbt_hi[:, c, :],
                                 start=False, stop=(c == 3))

            # duplicate C into partitions 64-127 (partition-shifted copy)
            nc.vector.tensor_copy(g3b_v[64:128, :], g3b_v[0:64, :])

            # ---- PW in [s, b] layout (no transposes) ----
            db_sb = pool.tile([128, 2 * B], BF16, tag="db", name="db")
            nc.vector.tensor_copy(db_sb[:], ps_db[:])
            # partition-swapped copy of the B half (tensor_tensor requires
            # same start partition on all APs; tensor_copy does not)
            dbsw = pool.tile([128, B], BF16, tag="dbsw", name="dbsw")
            nc.vector.tensor_copy(dbsw[0:64, :], db_sb[64:128, B:2 * B])
            nc.vector.tensor_copy(dbsw[64:128, :], db_sb[0:64, B:2 * B])
            ptA = pool.tile([128, B], BF16, tag="ptA", name="ptA")
            ptC2 = pool.tile([128, B], BF16, tag="ptC2", name="ptC2")
            nc.vector.tensor_mul(ptA[:], db_sb[:, 0:B], db_sb[:, B:2 * B])
            nc.vector.tensor_mul(ptC2[:], db_sb[:, 0:B], dbsw[:])

            # ---- S4 per bank: out[b, j] = ptA^T G3a + ptC2^T G3b2 ----
            out_lo = pool.tile([128, 512], BF16, tag="outlo", name="outlo")
            out_hi = pool.tile([128, 512], BF16, tag="outhi", name="outhi")
            stores = []
            nc.tensor.matmul(ps_out_lo[:], ptA[:], g3a_v[:, 0:512],
                             start=True, stop=False)
            nc.tensor.matmul(ps_out_lo[:], ptC2[:], g3b_v[:, 0:512],
                             start=False, stop=True)
            cp_lo = nc.scalar.copy(out_lo[:], ps_out_lo[:])
            stores.append(nc.sync.dma_start(out[:, :256],
                                            out_lo.bitcast(F32)[:, :]))
            nc.tensor.matmul(ps_out_hi[:], ptA[:], g3a_v[:, 512:1024],
                             start=True, stop=False)
            last_mm = nc.tensor.matmul(ps_out_hi[:], ptC2[:],
                                       g3b_v[:, 512:1024],
                                       start=False, stop=True)
            cp_hi = nc.vector.tensor_copy(out_hi[:], ps_out_hi[:])
            stores.append(nc.scalar.dma_start(out[:, 256:],
                                              out_hi.bitcast(F32)[:, :]))

            # ---- tail: absorb every outstanding tick into SP's clock ----
            prev = None
            for dep in [*in_dmas, memset_h, *stores, last_mm, cp_lo, cp_hi]:
                dr = nc.sync.drain(fusable=False)
                add_dep_helper(dr.ins, dep.ins, sync=True,
                               reason="tail: absorb tick into SP clock")
                if prev is not None:
                    add_dep_helper(dr.ins, prev.ins, sync=False,
                                   reason="tail: keep drain chain ordered")
                prev = dr

    return nc


def _bf16_pack(a):
    """float32 (P, W) -> bf16 packed two-per-word as float32 (P, W//2)."""
    bf = np.ascontiguousarray(np.asarray(a, np.float32).astype(ml_dtypes.bfloat16))
    return bf.view(np.uint8).reshape(bf.shape[0], -1).view(np.float32)


def _partition_pack(a):
    """(n*128, W) -> (128, n*W): row p = concat of chunk rows p."""
    r, w = a.shape
    n = r // 128
    return np.ascontiguousarray(
        a.reshape(n, 128, w).transpose(1, 0, 2).reshape(128, n * w))


def _constants():
    """Per-core CC [N, S], G3a [128, N], G3b2 [128, N] float32."""
    j = np.arange(N, dtype=np.float64)
    alt = np.cos(np.pi * j)                     # (-1)^j
    ccs, g3as, g3bs = [], [], []
    for c in range(N_CORES):
        f = np.arange(c * FPC, (c + 1) * FPC, dtype=np.float64)
        ang = 2.0 * np.pi * np.outer(j, f) / N             # (j, t)
        cc_re = np.cos(ang)
        cc_im = -np.sin(ang)
        angT = ang.T                                        # (t, j)
        w = 4.0 / N
        A = w * np.cos(angT)                                # m0 rows
        Bm = -w * np.cos(angT)                              # m1 rows
        C = -w * np.sin(angT)                               # mC rows
        if c == 0:
            cc_im[:, 0] = alt                               # f=512 cos column
            A[0, :] = 2.0 / N                               # m0 = D0*B0
            Bm[0, :] = (2.0 / N) * alt                      # m1 = D512*B512
            C[0, :] = 0.0
        cc_full = np.concatenate([cc_re, cc_im], axis=1)    # (N, 128)
        ccs.append(np.ascontiguousarray(cc_full, np.float32))
        g3as.append(np.ascontiguousarray(
            np.concatenate([A, Bm], axis=0), np.float32))       # (128, N)
        g3bs.append(np.ascontiguousarray(
            np.concatenate([C, C], axis=0), np.float32))        # (128, N)
    return ccs, g3as, g3bs


def kernel(des, body, kernel):
    global LAST_RESULT
    K = np.asarray(kernel, dtype=np.float32)
    des = np.asarray(des, dtype=np.float32)
    body = np.asarray(body, dtype=np.float32)

    # K^T as bf16 blocks: block (c, h) = K^T[c*128:(c+1)*128, h*512:(h+1)*512]
    ktb = K.T.astype(ml_dtypes.bfloat16)                # (1024 j, 1024 k)
    def ktpk(c, h):
        blk = np.ascontiguousarray(
            ktb[c * 128:(c + 1) * 128, h * 512:(h + 1) * 512], np.float32)
        return _bf16_pack(blk)                          # (128, 256) words

    id_pk = _bf16_pack(np.eye(128, dtype=np.float32))   # (128, 64) words
    dbt_np = np.concatenate([des.T, body.T], axis=1)    # (1024, 256)
    dbt_pk = _partition_pack(_bf16_pack(dbt_np))        # (128, 1024) words

    ccs, g3as, g3bs = _constants()
    in_maps = []
    for c in range(N_CORES):
        cc_pk = _partition_pack(_bf16_pack(ccs[c]))     # (128, 512) words
        m = {
            "sp1": np.ascontiguousarray(np.concatenate(
                [id_pk, ktpk(0, 0), ktpk(1, 0), ktpk(2, 0), ktpk(3, 0)],
                axis=1)),
            "sp2": np.ascontiguousarray(np.concatenate(
                [ktpk(0, 1), ktpk(1, 1), ktpk(2, 1), ktpk(3, 1)], axis=1)),
            "ac1": np.ascontiguousarray(np.concatenate(
                [ktpk(4, 0), ktpk(5, 0), ktpk(6, 0), ktpk(7, 0)], axis=1)),
            "ac2": np.ascontiguousarray(np.concatenate(
                [ktpk(4, 1), ktpk(5, 1), ktpk(6, 1), ktpk(7, 1),
                 dbt_pk], axis=1)),
            "cc": cc_pk,
            "g3a": np.ascontiguousarray(_bf16_pack(g3as[c])),
            "g3b": np.ascontiguousarray(_bf16_pack(g3bs[c][0:64])),
        }
        in_maps.append(m)

    if "nc" not in _nc_cache:
        _nc_cache["nc"] = _build_nc()
    nc = _nc_cache["nc"]

    res = run_bass_kernel_spmd(nc, in_maps, list(range(N_CORES)))
    LAST_RESULT = res
    out = np.zeros((B, N), dtype=np.float32)
    for r in res.results:
        w = np.ascontiguousarray(np.asarray(r["out"], np.float32))
        bf = w.view(np.uint8).reshape(B, -1).view(ml_dtypes.bfloat16)
        out += bf.astype(np.float32)
    return out
